# revision 1
# baseline (speedup 1.0000x reference)
"""GraphSAGE (mean) 3-layer encoder on 8 Trainium2 NeuronCores.

Strategy (graph/data parallel, per sharding hint):
  - Nodes sharded contiguously across 8 cores (12500/core, padded to
    12544 = 98*128 "slots"); per-core nodes permuted by in-degree.
  - Edges routed by dst core on the host. Per 128-slot dst block, edges
    are grouped by src bucket (4 slices of 25088 feature-table rows, so
    dma_gather's int16 indices reach every row) and chunked into groups
    of 128; host ships the int16 gather stream plus per-chunk dst-slot /
    (1/deg) vectors.
  - Per layer on device: dma_gather of src rows (bf16) -> one-hot
    selector built by one DVE tensor_scalar per chunk (iota == dstslot,
    scaled by 1/deg) -> PE matmul (gathered^T @ onehot) accumulates the
    mean-aggregated neighborhood feature-major in PSUM -> fp32r 512-wide
    dense matmuls (self + neigh in one PSUM) -> bias+ReLU (ACT) ->
    PE transpose -> L2 norm + residual (ACT/DVE) -> bf16 cast-DMA ->
    AllGather rebuilds the global feature table.
"""

import math
import sys

import numpy as np

for _p in ("/opt/trn_rl_repo", "/root/.axon_site/_ro/trn_rl_repo"):
    if _p not in sys.path:
        sys.path.append(_p)

import concourse.bacc as bacc  # noqa: E402
import concourse.bass as bass  # noqa: E402
import concourse.mybir as mybir  # noqa: E402
import concourse.tile as tile  # noqa: E402
from concourse import bass_utils  # noqa: E402
from concourse.masks import make_identity  # noqa: E402

M = 8  # cores
D = 128
P = 128
NBUC = 4  # src buckets (int16 index range)
GRP = 4  # dst blocks per dense group

LAST_EXEC_NS = None  # set by kernel() when _trace=True


def _host_prep(x, src, dst, n_nodes):
    N = n_nodes
    NPC = math.ceil(N / M)
    SLOTS = math.ceil(NPC / P) * P
    NBLK = SLOTS // P
    TBL = M * SLOTS
    BUC = TBL // NBUC
    assert BUC * NBUC == TBL and BUC <= 32768

    x = np.asarray(x).astype(np.int64)
    src = np.asarray(src).astype(np.int64)
    dst = np.asarray(dst).astype(np.int64)

    deg = np.bincount(dst, minlength=N)
    core_of_node = np.minimum(np.arange(N) // NPC, M - 1)
    perm = np.empty(N, np.int64)
    for c in range(M):
        lo, hi = c * NPC, min((c + 1) * NPC, N)
        nodes = np.arange(lo, hi)
        order = np.argsort(deg[nodes], kind="stable")
        r = np.empty(len(nodes), np.int64)
        r[order] = np.arange(len(nodes))
        perm[nodes] = r
    gslot = core_of_node * SLOTS + perm

    ecore = core_of_node[dst]
    # per-core edge arrays sorted by (block, bucket, slot)
    cores_edges = []
    cnt_cjb = np.zeros((M, NBLK, NBUC), np.int64)
    for c in range(M):
        sel = ecore == c
        dslot = perm[dst[sel]]
        sg = gslot[src[sel]]
        buc = sg // BUC
        blk = dslot // P
        o = np.lexsort((dslot, buc, blk))
        dslot, sg, buc, blk = dslot[o], sg[o], buc[o], blk[o]
        cores_edges.append((dslot, sg, buc, blk))
        np.add.at(cnt_cjb[c], (blk, buc), 1)

    C_jb = np.ceil(cnt_cjb / P).astype(np.int64).max(axis=0)  # [NBLK, NBUC]
    # a block with zero chunks still needs one (zero) chunk for neigh=0
    for j in range(NBLK):
        if C_jb[j].sum() == 0:
            C_jb[j, 0] = 1

    # balance blocks into groups of <= GRP, minimizing max total chunks
    C_j = C_jb.sum(axis=1)
    ngroups = math.ceil(NBLK / GRP)
    order = np.argsort(-C_j, kind="stable")
    gsum = np.zeros(ngroups, np.int64)
    gcnt = np.zeros(ngroups, np.int64)
    groups = [[] for _ in range(ngroups)]
    for j in order:
        cand = [g for g in range(ngroups) if gcnt[g] < GRP]
        g = min(cand, key=lambda q: gsum[q])
        groups[g].append(int(j))
        gsum[g] += C_j[j]
        gcnt[g] += 1
    groups = [sorted(g) for g in groups]

    # chunk-column layout: for g, for b, for j in g: C_jb[j, b] chunks
    chcol = np.zeros((NBLK, NBUC), np.int64)  # first global chunk of (j, b)
    calls = []  # per group: list of (b, ch0, ch1)
    blockchunks = {j: [] for j in range(NBLK)}
    pos = 0
    for g in groups:
        gc = []
        for b in range(NBUC):
            ch0 = pos
            for j in g:
                chcol[j, b] = pos
                for ci in range(int(C_jb[j, b])):
                    blockchunks[j].append((b, pos + ci))
                pos += int(C_jb[j, b])
            if pos > ch0:
                gc.append((b, ch0, pos))
        calls.append(gc)
    NCH = pos
    NIDX = NCH * P

    per_core = []
    for c in range(M):
        dslot, sg, buc, blk = cores_edges[c]
        # rank within (block, bucket)
        starts = np.zeros((NBLK, NBUC), np.int64)
        flat = (blk * NBUC + buc).astype(np.int64)
        cnts = cnt_cjb[c].reshape(-1)
        st = np.zeros(NBLK * NBUC, np.int64)
        st[1:] = np.cumsum(cnts)[:-1]
        rank = np.arange(len(dslot)) - st[flat]
        ch = chcol[blk, buc] + rank // P
        pp = rank % P
        idxs = np.zeros(NIDX, np.int16)
        dstloc = np.full((P, NCH), 255.0, np.float32)
        wvec = np.zeros((P, NCH), np.float32)
        idxs[ch * P + pp] = (sg - buc * BUC).astype(np.int16)
        dstloc[pp, ch] = (dslot % P).astype(np.float32)
        lo = c * NPC
        invd = 1.0 / np.maximum(deg, 1.0)
        # src-side weight is 1/deg of the *dst* node
        # recover dst node id: slot -> node
        node_of_slot = np.zeros(SLOTS, np.int64)
        nodes = np.arange(lo, min((c + 1) * NPC, N))
        node_of_slot[perm[nodes]] = nodes
        wvec[pp, ch] = invd[node_of_slot[dslot]].astype(np.float32)

        idx16 = idxs.reshape(NIDX // 16, 16).T.copy()  # [16, NIDX/16]
        idx_full = np.tile(idx16, (8, 1))  # [128, NIDX/16]

        x_slot = np.zeros(SLOTS, np.int32)
        x_slot[perm[nodes]] = x[nodes].astype(np.int32)
        xidx = x_slot.reshape(NBLK, P).T.copy()

        per_core.append(
            {"gidx": idx_full, "dstloc": dstloc, "wvec": wvec, "xidx": xidx}
        )

    meta = {
        "NPC": NPC,
        "SLOTS": SLOTS,
        "NBLK": NBLK,
        "TBL": TBL,
        "BUC": BUC,
        "groups": groups,
        "calls": calls,
        "blockchunks": blockchunks,
        "NCH": NCH,
        "NIDX": NIDX,
        "gslot": gslot,
        "pad_frac": NCH * P / max(1, len(src)) * M / M,
    }
    return per_core, meta


def _build_program(meta, V, L, single_core=False):
    SLOTS, NBLK, TBL, BUC = meta["SLOTS"], meta["NBLK"], meta["TBL"], meta["BUC"]
    groups, calls, blockchunks = meta["groups"], meta["calls"], meta["blockchunks"]
    NCH, NIDX = meta["NCH"], meta["NIDX"]
    CBMAX = max((ch1 - ch0) for gc in calls for (_, ch0, ch1) in gc)

    f32, f32r, bf16 = mybir.dt.float32, mybir.dt.float32r, mybir.dt.bfloat16
    i16, i32 = mybir.dt.int16, mybir.dt.int32

    nc = bacc.Bacc(
        "TRN2",
        target_bir_lowering=False,
        debug=False,
        enable_asserts=False,
        num_devices=1 if single_core else M,
    )

    gidx_d = nc.dram_tensor("gidx", [P, NIDX // 16], i16, kind="ExternalInput")
    dstloc_d = nc.dram_tensor("dstloc", [P, NCH], f32, kind="ExternalInput")
    wvec_d = nc.dram_tensor("wvec", [P, NCH], f32, kind="ExternalInput")
    xidx_d = nc.dram_tensor("xidx", [P, NBLK], i32, kind="ExternalInput")
    emb_d = nc.dram_tensor("emb", [V, D], f32, kind="ExternalInput")
    ws_d = nc.dram_tensor("ws", [L, D, D], f32, kind="ExternalInput")
    wn_d = nc.dram_tensor("wn", [L, D, D], f32, kind="ExternalInput")
    bias_d = nc.dram_tensor("bias", [L, D], f32, kind="ExternalInput")
    hout_d = nc.dram_tensor("hout", [SLOTS, D], f32, kind="ExternalOutput")

    h_shard = nc.dram_tensor("h_shard", [SLOTS, D], bf16, kind="Internal")
    h_full = nc.dram_tensor(
        "h_full", [TBL, D], bf16, kind="Internal", addr_space="Shared"
    )

    rg = [list(range(M))]

    with tile.TileContext(nc) as tc:
        with (
            tc.tile_pool(name="const", bufs=1) as cpool,
            tc.tile_pool(name="state", bufs=1) as spool,
            tc.tile_pool(name="gath", bufs=6) as gpool,
            tc.tile_pool(name="oh", bufs=4) as ohpool,
            tc.tile_pool(name="fm", bufs=2) as fmpool,
            tc.tile_pool(name="small", bufs=3) as smpool,
            tc.tile_pool(name="ps_agg", bufs=2, space="PSUM") as ps_agg,
            tc.tile_pool(name="ps_tp", bufs=2, space="PSUM") as ps_tp,
            tc.tile_pool(name="ps_nm", bufs=2, space="PSUM") as ps_nm,
            tc.tile_pool(name="ps_d", bufs=2, space="PSUM") as ps_d,
        ):
            # ---- constants ----
            ident_f = cpool.tile([P, P], f32, tag="ident_f")
            make_identity(nc, ident_f[:])
            iota_bf = cpool.tile([P, P], bf16, tag="iota_bf")
            nc.gpsimd.iota(
                iota_bf[:],
                pattern=[[1, P]],
                base=0,
                channel_multiplier=0,
                allow_small_or_imprecise_dtypes=True,
            )

            gidx_sb = cpool.tile([P, NIDX // 16], i16, tag="gidx")
            nc.sync.dma_start(gidx_sb[:], gidx_d[:, :])
            dstloc_sb = cpool.tile([P, NCH], f32, tag="dstloc")
            nc.sync.dma_start(dstloc_sb[:], dstloc_d[:, :])
            wvec_sb = cpool.tile([P, NCH], f32, tag="wvec")
            nc.sync.dma_start(wvec_sb[:], wvec_d[:, :])
            xidx_sb = cpool.tile([P, NBLK], i32, tag="xidx")
            nc.sync.dma_start(xidx_sb[:], xidx_d[:, :])

            w_sb = []
            for l in range(L):
                wsf = cpool.tile([P, D], f32, tag=f"wsf{l}")
                wnf = cpool.tile([P, D], f32, tag=f"wnf{l}")
                nc.sync.dma_start(wsf[:], ws_d[l, :, :])
                nc.sync.dma_start(wnf[:], wn_d[l, :, :])
                ws = cpool.tile([P, D], f32r, tag=f"ws{l}")
                wn = cpool.tile([P, D], f32r, tag=f"wn{l}")
                nc.scalar.copy(ws[:], wsf[:])
                nc.scalar.copy(wn[:], wnf[:])
                w_sb.append((ws, wn))
            b_sb = cpool.tile([P, L], f32, tag="bias")
            for l in range(L):
                nc.sync.dma_start(b_sb[:, l : l + 1], bias_d[l, :, None])

            # ---- embedding lookup (128 rows per call, int32 indices) ----
            e_sb = spool.tile([P, NBLK * D], f32, tag="e")
            for j in range(NBLK):
                nc.gpsimd.indirect_dma_start(
                    out=e_sb[:, j * D : (j + 1) * D],
                    out_offset=None,
                    in_=emb_d[:, :],
                    in_offset=bass.IndirectOffsetOnAxis(
                        ap=xidx_sb[:, j : j + 1], axis=0
                    ),
                )

            h_sb = spool.tile([P, NBLK * D], f32, tag="h")

            shard_v = h_shard.ap().rearrange("(j p) f -> p j f", p=P)

            def store_table(src_tile):
                sv = src_tile[:].rearrange("p (j f) -> p j f", f=D)
                nc.gpsimd.dma_start(out=shard_v, in_=sv)  # SWDGE cast
                if single_core:
                    return  # timing-only variant: no collective
                nc.gpsimd.collective_compute(
                    "AllGather",
                    mybir.AluOpType.bypass,
                    replica_groups=rg,
                    ins=[h_shard[:, :]],
                    outs=[h_full[:, :]],
                )

            store_table(e_sb)

            # ---- layers ----
            for l in range(L):
                cur = e_sb if l == 0 else h_sb
                ws, wn = w_sb[l]
                for gi, grp in enumerate(groups):
                    gtiles = {}
                    for (b, ch0, ch1) in calls[gi]:
                        gt = gpool.tile([P, CBMAX, D], bf16, tag="gath")
                        ni = (ch1 - ch0) * P
                        nc.gpsimd.dma_gather(
                            gt[:, 0 : ch1 - ch0, :],
                            h_full[b * BUC : (b + 1) * BUC, :],
                            gidx_sb[:, ch0 * 8 : ch1 * 8],
                            ni,
                            ni,
                            D,
                            single_packet=False,
                        )
                        gtiles[b] = (gt, ch0)
                    nfm = fmpool.tile([P, GRP * D], f32r, tag="nfm")
                    hfm = fmpool.tile([P, GRP * D], f32r, tag="hfm")
                    for bi, j in enumerate(grp):
                        chunks = blockchunks[j]
                        pa = ps_agg.tile([P, P], f32, tag="agg")
                        nch = len(chunks)
                        for ci, (b, ch) in enumerate(chunks):
                            gt, ch0 = gtiles[b]
                            oh = ohpool.tile([P, P], bf16, tag="oh")
                            nc.vector.tensor_scalar(
                                oh[:],
                                iota_bf[:],
                                dstloc_sb[:, ch : ch + 1],
                                wvec_sb[:, ch : ch + 1],
                                mybir.AluOpType.is_equal,
                                mybir.AluOpType.mult,
                            )
                            nc.tensor.matmul(
                                pa[:],
                                gt[:, ch - ch0, :],
                                oh[:],
                                start=(ci == 0),
                                stop=(ci == nch - 1),
                            )
                        # pa is feature-major mean-aggregated neigh
                        nc.scalar.copy(nfm[:, bi * D : (bi + 1) * D], pa[:])
                        pt = ps_tp.tile([P, P], f32, tag="tp")
                        nc.tensor.transpose(
                            pt[:], cur[:, j * D : (j + 1) * D], ident_f[:]
                        )
                        nc.scalar.copy(hfm[:, bi * D : (bi + 1) * D], pt[:])
                    gw = len(grp) * D
                    pd = ps_d.tile([P, GRP * D], f32, tag="d")
                    nc.tensor.matmul(
                        pd[:, 0:gw], ws[:], hfm[:, 0:gw], start=True, stop=False
                    )
                    nc.tensor.matmul(
                        pd[:, 0:gw], wn[:], nfm[:, 0:gw], start=False, stop=True
                    )
                    hpre = fmpool.tile([P, GRP * D], f32, tag="hpre")
                    nc.scalar.activation(
                        hpre[:, 0:gw],
                        pd[:, 0:gw],
                        mybir.ActivationFunctionType.Relu,
                        bias=b_sb[:, l : l + 1],
                    )
                    for bi, j in enumerate(grp):
                        pn = ps_nm.tile([P, P], f32, tag="nm")
                        nc.tensor.transpose(
                            pn[:], hpre[:, bi * D : (bi + 1) * D], ident_f[:]
                        )
                        sq = smpool.tile([P, D], f32, tag="sq")
                        ss = smpool.tile([P, 1], f32, tag="ss")
                        nc.scalar.activation(
                            sq[:],
                            pn[:],
                            mybir.ActivationFunctionType.Square,
                            accum_out=ss[:],
                        )
                        nrm = smpool.tile([P, 1], f32, tag="nrm")
                        nc.scalar.sqrt(nrm[:], ss[:])
                        nc.vector.tensor_scalar_max(nrm[:], nrm[:], 1e-12)
                        inv = smpool.tile([P, 1], f32, tag="inv")
                        nc.vector.reciprocal(inv[:], nrm[:])
                        htmp = smpool.tile([P, D], f32, tag="htmp")
                        nc.vector.tensor_scalar(
                            htmp[:], pn[:], inv[:], None, mybir.AluOpType.mult
                        )
                        nc.vector.tensor_tensor(
                            out=h_sb[:, j * D : (j + 1) * D],
                            in0=htmp[:],
                            in1=e_sb[:, j * D : (j + 1) * D],
                            op=mybir.AluOpType.add,
                        )
                if l < L - 1:
                    store_table(h_sb)

            hout_v = hout_d.ap().rearrange("(j p) f -> p j f", p=P)
            h_v = h_sb[:].rearrange("p (j f) -> p j f", f=D)
            nc.sync.dma_start(hout_v, h_v)

    nc.compile()
    return nc


def kernel(x, src, dst, emb, Ws, Wn, b, _trace=False):
    x = np.asarray(x)
    src = np.asarray(src)
    dst = np.asarray(dst)
    emb = np.ascontiguousarray(np.asarray(emb, dtype=np.float32))
    Ws = np.ascontiguousarray(np.asarray(Ws, dtype=np.float32))
    Wn = np.ascontiguousarray(np.asarray(Wn, dtype=np.float32))
    b = np.ascontiguousarray(np.asarray(b, dtype=np.float32))
    N = x.shape[0]
    V, _ = emb.shape
    L = Ws.shape[0]

    per_core, meta = _host_prep(x, src, dst, N)
    nc = _build_program(meta, V, L)

    in_maps = []
    for c in range(M):
        pc = per_core[c]
        in_maps.append(
            {
                "gidx": np.ascontiguousarray(pc["gidx"]),
                "dstloc": np.ascontiguousarray(pc["dstloc"]),
                "wvec": np.ascontiguousarray(pc["wvec"]),
                "xidx": np.ascontiguousarray(pc["xidx"]),
                "emb": emb,
                "ws": Ws,
                "wn": Wn,
                "bias": b,
            }
        )

    res = bass_utils.run_bass_kernel_spmd(
        nc, in_maps, core_ids=list(range(M)), trace=_trace
    )
    global LAST_EXEC_NS
    LAST_EXEC_NS = res.exec_time_ns
    outs = [np.asarray(r["hout"], dtype=np.float32) for r in res.results]
    big = np.concatenate(outs, axis=0)
    return big[meta["gslot"]]



# revision 16
# speedup vs baseline: 1.0476x; 1.0476x over previous
"""GraphSAGE (mean) 3-layer encoder on 8 Trainium2 NeuronCores.

Strategy (graph/data parallel, per sharding hint):
  - Nodes sharded contiguously across 8 cores (12500/core, padded to
    12544 = 98*128 "slots"); per-core nodes permuted by in-degree.
  - Edges routed by dst core on the host. Per 128-slot dst block, edges
    are grouped by src bucket (4 slices of 25088 feature-table rows, so
    dma_gather's int16 indices reach every row) and chunked into groups
    of 128; host ships the int16 gather stream plus per-chunk dst-slot
    vectors and a per-slot 1/deg vector.
  - Per layer on device: dma_gather of src rows (bf16) -> pure 0/1
    one-hot selectors built 4 chunks at a time by ONE DVE tensor_tensor
    (is_equal of iota vs broadcast dst-slot columns; tensor_tensor runs
    in 1-port mode so it never takes the SBUF port pair away from the
    Q7 SWDGE descriptor generator) -> PE matmul (gathered^T @ onehot)
    accumulates the SUM-aggregated neighborhood feature-major in PSUM ->
    fp32r 512-wide dense matmuls (self and neigh in separate PSUMs) ->
    bias on self path (ACT, feature-major) -> PE transposes to
    node-major -> 1/deg scale on neigh (ACT per-partition) + add + ReLU
    -> L2 norm + residual -> bf16 cast-DMA -> AllGather rebuilds the
    global feature table.
"""

import math
import sys

import numpy as np
import ml_dtypes

for _p in ("/opt/trn_rl_repo", "/root/.axon_site/_ro/trn_rl_repo"):
    if _p not in sys.path:
        sys.path.append(_p)

import concourse.bacc as bacc  # noqa: E402
import concourse.bass as bass  # noqa: E402
import concourse.mybir as mybir  # noqa: E402
import concourse.tile as tile  # noqa: E402
from concourse import bass_utils  # noqa: E402
from concourse.masks import make_identity  # noqa: E402

M = 8  # cores
D = 128
P = 128
NBUC = 4  # src buckets (int16 index range)
GRP = 4  # dst blocks per dense group

LAST_EXEC_NS = None  # set by kernel() when _trace=True


def _host_prep(x, src, dst, n_nodes):
    N = n_nodes
    NPC = math.ceil(N / M)
    SLOTS = math.ceil(NPC / P) * P
    NBLK = SLOTS // P
    TBL = M * SLOTS
    BUC = TBL // NBUC
    assert BUC * NBUC == TBL and BUC <= 32768

    x = np.asarray(x).astype(np.int64)
    src = np.asarray(src).astype(np.int64)
    dst = np.asarray(dst).astype(np.int64)

    deg = np.bincount(dst, minlength=N)
    core_of_node = np.minimum(np.arange(N) // NPC, M - 1)
    perm = np.empty(N, np.int64)
    for c in range(M):
        lo, hi = c * NPC, min((c + 1) * NPC, N)
        nodes = np.arange(lo, hi)
        order = np.argsort(deg[nodes], kind="stable")
        r = np.empty(len(nodes), np.int64)
        r[order] = np.arange(len(nodes))
        perm[nodes] = r
    gslot = core_of_node * SLOTS + perm

    ecore = core_of_node[dst]
    # per-core edge arrays sorted by (block, bucket, slot)
    cores_edges = []
    cnt_cjb = np.zeros((M, NBLK, NBUC), np.int64)
    for c in range(M):
        sel = ecore == c
        dslot = perm[dst[sel]]
        sg = gslot[src[sel]]
        buc = sg // BUC
        blk = dslot // P
        o = np.lexsort((dslot, buc, blk))
        dslot, sg, buc, blk = dslot[o], sg[o], buc[o], blk[o]
        cores_edges.append((dslot, sg, buc, blk))
        np.add.at(cnt_cjb[c], (blk, buc), 1)

    C_jb = np.ceil(cnt_cjb / P).astype(np.int64).max(axis=0)  # [NBLK, NBUC]
    # a block with zero chunks still needs one (zero) chunk for neigh=0
    for j in range(NBLK):
        if C_jb[j].sum() == 0:
            C_jb[j, 0] = 1

    # balance blocks into groups of <= GRP, minimizing max total chunks
    C_j = C_jb.sum(axis=1)
    ngroups = math.ceil(NBLK / GRP)
    order = np.argsort(-C_j, kind="stable")
    gsum = np.zeros(ngroups, np.int64)
    gcnt = np.zeros(ngroups, np.int64)
    groups = [[] for _ in range(ngroups)]
    for j in order:
        cand = [g for g in range(ngroups) if gcnt[g] < GRP]
        g = min(cand, key=lambda q: gsum[q])
        groups[g].append(int(j))
        gsum[g] += C_j[j]
        gcnt[g] += 1
    groups = [sorted(g) for g in groups]

    # gather-stream layout: for g, for b, for j in g: C_jb[j, b] chunks
    chcol = np.zeros((NBLK, NBUC), np.int64)  # first stream chunk of (j, b)
    calls = []  # per group: list of (b, ch0, ch1)
    pos = 0
    for g in groups:
        gc = []
        for b in range(NBUC):
            ch0 = pos
            for j in g:
                chcol[j, b] = pos
                pos += int(C_jb[j, b])
            if pos > ch0:
                gc.append((b, ch0, pos))
        calls.append(gc)
    NCH = pos
    NIDX = NCH * P

    # one-hot column layout: block-major, 4-aligned per block (quad builds).
    # blockchunks[j] = [(bucket, stream_chunk, oh_col), ...]
    dcol0 = np.zeros(NBLK, np.int64)
    blockchunks = {j: [] for j in range(NBLK)}
    dpos = 0
    for j in range(NBLK):
        dcol0[j] = dpos
        i = 0
        for b in range(NBUC):
            for ci in range(int(C_jb[j, b])):
                blockchunks[j].append((b, int(chcol[j, b]) + ci, dpos + i))
                i += 1
        dpos += 4 * math.ceil(i / 4)
    NCHD = dpos

    per_core = []
    for c in range(M):
        dslot, sg, buc, blk = cores_edges[c]
        # rank within (block, bucket)
        flat = (blk * NBUC + buc).astype(np.int64)
        cnts = cnt_cjb[c].reshape(-1)
        st = np.zeros(NBLK * NBUC, np.int64)
        st[1:] = np.cumsum(cnts)[:-1]
        rank = np.arange(len(dslot)) - st[flat]
        ch = chcol[blk, buc] + rank // P
        pp = rank % P
        idxs = np.zeros(NIDX, np.int16)
        dstloc = np.full((P, NCHD), 255.0, ml_dtypes.bfloat16)
        idxs[ch * P + pp] = (sg - buc * BUC).astype(np.int16)
        # one-hot column of each edge: block-major numbering
        cib = np.zeros((NBLK, NBUC), np.int64)  # chunk-in-block start of (j,b)
        for j in range(NBLK):
            acc = 0
            for b in range(NBUC):
                cib[j, b] = acc
                acc += int(C_jb[j, b])
        dc = dcol0[blk] + cib[blk, buc] + rank // P
        dstloc[pp, dc] = (dslot % P).astype(np.float32)
        lo = c * NPC
        invd = 1.0 / np.maximum(deg, 1.0)
        # per-slot 1/deg vector (node-major; pad slots get 1.0)
        nodes = np.arange(lo, min((c + 1) * NPC, N))
        node_of_slot = np.full(SLOTS, -1, np.int64)
        node_of_slot[perm[nodes]] = nodes
        invd_slot = np.ones(SLOTS, np.float32)
        real = node_of_slot >= 0
        invd_slot[real] = invd[node_of_slot[real]].astype(np.float32)
        invd_sb = invd_slot.reshape(NBLK, P).T.copy()  # [P, NBLK]

        idx16 = idxs.reshape(NIDX // 16, 16).T.copy()  # [16, NIDX/16]
        idx_full = np.tile(idx16, (8, 1))  # [128, NIDX/16]

        x_slot = np.zeros(SLOTS, np.int32)
        x_slot[perm[nodes]] = x[nodes].astype(np.int32)
        xidx = x_slot.reshape(NBLK, P).T.copy()

        per_core.append(
            {"gidx": idx_full, "dstloc": dstloc, "invd": invd_sb, "xidx": xidx}
        )

    meta = {
        "NPC": NPC,
        "SLOTS": SLOTS,
        "NBLK": NBLK,
        "TBL": TBL,
        "BUC": BUC,
        "groups": groups,
        "calls": calls,
        "blockchunks": blockchunks,
        "dcol0": dcol0,
        "NCH": NCH,
        "NCHD": NCHD,
        "NIDX": NIDX,
        "gslot": gslot,
    }
    return per_core, meta


def _build_program(meta, V, L, single_core=False):
    SLOTS, NBLK, TBL, BUC = meta["SLOTS"], meta["NBLK"], meta["TBL"], meta["BUC"]
    groups, calls, blockchunks = meta["groups"], meta["calls"], meta["blockchunks"]
    dcol0 = meta["dcol0"]
    NCH, NCHD, NIDX = meta["NCH"], meta["NCHD"], meta["NIDX"]
    CBMAX = max((ch1 - ch0) for gc in calls for (_, ch0, ch1) in gc)

    f32, f32r, bf16 = mybir.dt.float32, mybir.dt.float32r, mybir.dt.bfloat16
    i16, i32 = mybir.dt.int16, mybir.dt.int32

    nc = bacc.Bacc(
        "TRN2",
        target_bir_lowering=False,
        debug=False,
        enable_asserts=False,
        num_devices=1 if single_core else M,
    )

    gidx_d = nc.dram_tensor("gidx", [P, NIDX // 16], i16, kind="ExternalInput")
    dstloc_d = nc.dram_tensor("dstloc", [P, NCHD], bf16, kind="ExternalInput")
    invd_d = nc.dram_tensor("invd", [P, NBLK], f32, kind="ExternalInput")
    xidx_d = nc.dram_tensor("xidx", [P, NBLK], i32, kind="ExternalInput")
    emb_d = nc.dram_tensor("emb", [V, D], f32, kind="ExternalInput")
    ws_d = nc.dram_tensor("ws", [L, D, D], f32, kind="ExternalInput")
    wn_d = nc.dram_tensor("wn", [L, D, D], f32, kind="ExternalInput")
    bias_d = nc.dram_tensor("bias", [L, D], f32, kind="ExternalInput")
    hout_d = nc.dram_tensor("hout", [SLOTS, D], f32, kind="ExternalOutput")

    h_shard = nc.dram_tensor("h_shard", [SLOTS, D], bf16, kind="Internal")
    h_full = nc.dram_tensor(
        "h_full", [TBL, D], bf16, kind="Internal", addr_space="Shared"
    )

    rg = [list(range(M))]

    with tile.TileContext(nc) as tc:
        with (
            tc.tile_pool(name="const", bufs=1) as cpool,
            tc.tile_pool(name="state", bufs=1) as spool,
            tc.tile_pool(name="gath", bufs=6) as gpool,
            tc.tile_pool(name="oh", bufs=12) as ohpool,
            tc.tile_pool(name="fm", bufs=2) as fmpool,
            tc.tile_pool(name="small", bufs=3) as smpool,
            tc.tile_pool(name="ps_blk", bufs=2, space="PSUM") as ps_blk,
            tc.tile_pool(name="ps_nm", bufs=2, space="PSUM") as ps_nm,
            tc.tile_pool(name="ps_d", bufs=2, space="PSUM") as ps_d,
        ):
            # ---- constants ----
            ident_f = cpool.tile([P, P], f32, tag="ident_f")
            make_identity(nc, ident_f[:])
            # iota repeated 4x along free dim: iota4[p, r*128 + c] = c
            iota4 = cpool.tile([P, 4 * P], bf16, tag="iota4")
            nc.gpsimd.iota(
                iota4[:].rearrange("p (r c) -> p r c", c=P),
                pattern=[[0, 4], [1, P]],
                base=0,
                channel_multiplier=0,
                allow_small_or_imprecise_dtypes=True,
            )

            gidx_sb = cpool.tile([P, NIDX // 16], i16, tag="gidx")
            nc.sync.dma_start(gidx_sb[:], gidx_d[:, :])
            dstloc_sb = cpool.tile([P, NCHD], bf16, tag="dstloc")
            nc.sync.dma_start(dstloc_sb[:], dstloc_d[:, :])
            invd_sb = cpool.tile([P, NBLK], f32, tag="invd")
            nc.sync.dma_start(invd_sb[:], invd_d[:, :])
            xidx_sb = cpool.tile([P, NBLK], i32, tag="xidx")
            nc.sync.dma_start(xidx_sb[:], xidx_d[:, :])

            w_sb = []
            for l in range(L):
                wsf = cpool.tile([P, D], f32, tag=f"wsf{l}")
                wnf = cpool.tile([P, D], f32, tag=f"wnf{l}")
                nc.sync.dma_start(wsf[:], ws_d[l, :, :])
                nc.sync.dma_start(wnf[:], wn_d[l, :, :])
                ws = cpool.tile([P, D], f32r, tag=f"ws{l}")
                wn = cpool.tile([P, D], f32r, tag=f"wn{l}")
                nc.scalar.copy(ws[:], wsf[:])
                nc.scalar.copy(wn[:], wnf[:])
                w_sb.append((ws, wn))
            b_sb = cpool.tile([P, L], f32, tag="bias")
            for l in range(L):
                nc.sync.dma_start(b_sb[:, l : l + 1], bias_d[l, :, None])

            # ---- embedding lookup (128 rows per call, int32 indices) ----
            e_sb = spool.tile([P, NBLK * D], f32, tag="e")
            for j in range(NBLK):
                nc.gpsimd.indirect_dma_start(
                    out=e_sb[:, j * D : (j + 1) * D],
                    out_offset=None,
                    in_=emb_d[:, :],
                    in_offset=bass.IndirectOffsetOnAxis(
                        ap=xidx_sb[:, j : j + 1], axis=0
                    ),
                )

            h_sb = spool.tile([P, NBLK * D], f32, tag="h")

            shard_v = h_shard.ap().rearrange("(j p) f -> p j f", p=P)

            def store_table(src_tile):
                sv = src_tile[:].rearrange("p (j f) -> p j f", f=D)
                nc.gpsimd.dma_start(out=shard_v, in_=sv)  # SWDGE cast
                if single_core:
                    return  # timing-only variant: no collective
                nc.gpsimd.collective_compute(
                    "AllGather",
                    mybir.AluOpType.bypass,
                    replica_groups=rg,
                    ins=[h_shard[:, :]],
                    outs=[h_full[:, :]],
                )

            store_table(e_sb)

            # ---- layers ----
            for l in range(L):
                cur = e_sb if l == 0 else h_sb
                ws, wn = w_sb[l]
                for gi, grp in enumerate(groups):
                    gtiles = {}
                    for (b, ch0, ch1) in calls[gi]:
                        gt = gpool.tile([P, CBMAX, D], bf16, tag="gath")
                        ni = (ch1 - ch0) * P
                        nc.gpsimd.dma_gather(
                            gt[:, 0 : ch1 - ch0, :],
                            h_full[b * BUC : (b + 1) * BUC, :],
                            gidx_sb[:, ch0 * 8 : ch1 * 8],
                            ni,
                            ni,
                            D,
                            single_packet=False,
                        )
                        gtiles[b] = (gt, ch0)
                    nfm = fmpool.tile([P, GRP * D], f32r, tag="nfm")
                    hfm = fmpool.tile([P, GRP * D], f32r, tag="hfm")
                    for bi, j in enumerate(grp):
                        chunks = blockchunks[j]
                        nch = len(chunks)
                        # one-hot selectors, 4 chunks per DVE op (1-port
                        # mode: never locks GpSimd out of the SBUF ports)
                        nquad = (nch + 3) // 4
                        d0 = int(dcol0[j])
                        ohqs = []
                        for t in range(nquad):
                            ohq = ohpool.tile([P, 4 * P], bf16, tag="oh")
                            s = d0 + 4 * t
                            nc.vector.tensor_tensor(
                                out=ohq[:].rearrange("p (r c) -> p r c", c=P),
                                in0=dstloc_sb[:, s : s + 4].to_broadcast(
                                    [P, 4, P]
                                ),
                                in1=iota4[:].rearrange("p (r c) -> p r c", c=P),
                                op=mybir.AluOpType.is_equal,
                            )
                            ohqs.append(ohq)
                        blk_ps = ps_blk.tile([P, 2 * P], f32, tag="blk")
                        pa, pt = blk_ps[:, 0:P], blk_ps[:, P : 2 * P]
                        for ci, (b, ch, dc) in enumerate(chunks):
                            gt, ch0 = gtiles[b]
                            q, r = (dc - d0) // 4, (dc - d0) % 4
                            nc.tensor.matmul(
                                pa,
                                gt[:, ch - ch0, :],
                                ohqs[q][:, r * P : (r + 1) * P],
                                start=(ci == 0),
                                stop=(ci == nch - 1),
                            )
                        # pa is feature-major sum-aggregated neigh
                        nc.scalar.copy(nfm[:, bi * D : (bi + 1) * D], pa)
                        nc.tensor.transpose(
                            pt, cur[:, j * D : (j + 1) * D], ident_f[:]
                        )
                        nc.scalar.copy(hfm[:, bi * D : (bi + 1) * D], pt)
                    gw = len(grp) * D
                    d_ps = ps_d.tile([P, 2 * GRP * D], f32, tag="d")
                    pdS = d_ps[:, 0 : GRP * D]
                    pdN = d_ps[:, GRP * D : 2 * GRP * D]
                    nc.tensor.matmul(
                        pdS[:, 0:gw], ws[:], hfm[:, 0:gw], start=True, stop=True
                    )
                    nc.tensor.matmul(
                        pdN[:, 0:gw], wn[:], nfm[:, 0:gw], start=True, stop=True
                    )
                    # self path + bias (feature-major, per-partition bias)
                    hbias = fmpool.tile([P, GRP * D], f32, tag="hbias")
                    nc.scalar.activation(
                        hbias[:, 0:gw],
                        pdS[:, 0:gw],
                        mybir.ActivationFunctionType.Identity,
                        bias=b_sb[:, l : l + 1],
                    )
                    nden = fmpool.tile([P, GRP * D], f32, tag="nden")
                    nc.scalar.copy(nden[:, 0:gw], pdN[:, 0:gw])
                    for bi, j in enumerate(grp):
                        nm_ps = ps_nm.tile([P, 2 * P], f32, tag="nm")
                        pnS, pnN = nm_ps[:, 0:P], nm_ps[:, P : 2 * P]
                        nc.tensor.transpose(
                            pnS, hbias[:, bi * D : (bi + 1) * D], ident_f[:]
                        )
                        nc.tensor.transpose(
                            pnN, nden[:, bi * D : (bi + 1) * D], ident_f[:]
                        )
                        # node-major: neigh * (1/deg), + self, ReLU
                        tn = smpool.tile([P, D], f32, tag="tn")
                        nc.scalar.activation(
                            tn[:],
                            pnN,
                            mybir.ActivationFunctionType.Identity,
                            scale=invd_sb[:, j : j + 1],
                        )
                        hp = smpool.tile([P, D], f32, tag="hp")
                        nc.vector.tensor_tensor(
                            out=hp[:], in0=pnS, in1=tn[:],
                            op=mybir.AluOpType.add,
                        )
                        hr = smpool.tile([P, D], f32, tag="hr")
                        sq_ss = smpool.tile([P, 1], f32, tag="ss")
                        nc.scalar.activation(
                            hr[:], hp[:], mybir.ActivationFunctionType.Relu
                        )
                        sq = smpool.tile([P, D], f32, tag="sq")
                        nc.scalar.activation(
                            sq[:],
                            hr[:],
                            mybir.ActivationFunctionType.Square,
                            accum_out=sq_ss[:],
                        )
                        nrm = smpool.tile([P, 1], f32, tag="nrm")
                        nc.scalar.sqrt(nrm[:], sq_ss[:])
                        nc.vector.tensor_scalar_max(nrm[:], nrm[:], 1e-12)
                        inv = smpool.tile([P, 1], f32, tag="inv")
                        nc.vector.reciprocal(inv[:], nrm[:])
                        htmp = smpool.tile([P, D], f32, tag="htmp")
                        nc.vector.tensor_tensor(
                            out=htmp[:],
                            in0=hr[:],
                            in1=inv[:, 0:1].to_broadcast([P, D]),
                            op=mybir.AluOpType.mult,
                        )
                        nc.vector.tensor_tensor(
                            out=h_sb[:, j * D : (j + 1) * D],
                            in0=htmp[:],
                            in1=e_sb[:, j * D : (j + 1) * D],
                            op=mybir.AluOpType.add,
                        )
                if l < L - 1:
                    store_table(h_sb)

            hout_v = hout_d.ap().rearrange("(j p) f -> p j f", p=P)
            h_v = h_sb[:].rearrange("p (j f) -> p j f", f=D)
            nc.sync.dma_start(hout_v, h_v)

    nc.compile()
    return nc


def kernel(x, src, dst, emb, Ws, Wn, b, _trace=False):
    x = np.asarray(x)
    src = np.asarray(src)
    dst = np.asarray(dst)
    emb = np.ascontiguousarray(np.asarray(emb, dtype=np.float32))
    Ws = np.ascontiguousarray(np.asarray(Ws, dtype=np.float32))
    Wn = np.ascontiguousarray(np.asarray(Wn, dtype=np.float32))
    b = np.ascontiguousarray(np.asarray(b, dtype=np.float32))
    N = x.shape[0]
    V, _ = emb.shape
    L = Ws.shape[0]

    per_core, meta = _host_prep(x, src, dst, N)
    nc = _build_program(meta, V, L)

    in_maps = []
    for c in range(M):
        pc = per_core[c]
        in_maps.append(
            {
                "gidx": np.ascontiguousarray(pc["gidx"]),
                "dstloc": np.ascontiguousarray(pc["dstloc"]),
                "invd": np.ascontiguousarray(pc["invd"]),
                "xidx": np.ascontiguousarray(pc["xidx"]),
                "emb": emb,
                "ws": Ws,
                "wn": Wn,
                "bias": b,
            }
        )

    res = bass_utils.run_bass_kernel_spmd(
        nc, in_maps, core_ids=list(range(M)), trace=_trace
    )
    global LAST_EXEC_NS
    LAST_EXEC_NS = res.exec_time_ns
    outs = [np.asarray(r["hout"], dtype=np.float32) for r in res.results]
    big = np.concatenate(outs, axis=0)
    return big[meta["gslot"]]


# revision 22
# speedup vs baseline: 2.2853x; 2.1814x over previous
"""GraphSAGE (mean) 3-layer encoder on 8 Trainium2 NeuronCores.

Strategy (graph/data parallel, per sharding hint):
  - Nodes sharded contiguously across 8 cores (12500/core, padded to
    12544 = 98*128 "slots"); per-core nodes permuted by in-degree.
  - Global feature table laid out half-major (half, core, slot) so each
    half of a core's shard AllGathers into a contiguous table range;
    the two AllGathers per layer overlap with compute (half-0 launches
    mid-layer, half-1 only blocks next layer's bucket-2/3 gathers).
  - Edges routed by dst core on the host. Per 128-slot dst block, edges
    are grouped by src bucket (4 slices of 25088 table rows so
    dma_gather's int16 indices reach every row) and packed densely into
    per-(group,bucket) segments using shared per-(block,bucket) slot
    allocations (max edge count over cores) -> ~6% padding. Chunks of
    128 gathered rows may straddle adjacent dst blocks; each (chunk,
    block) pair gets its own one-hot column set.
  - dma_gather calls round-robin over 4 SWDGE queues: descriptor
    generation parallelizes across Q7 cores (~4x).
  - Per layer on device: dma_gather of src rows (bf16) -> pure 0/1
    one-hot selectors built 4 columns at a time by ONE DVE
    tensor_tensor (is_equal vs broadcast dst-slot columns; 1-port mode,
    never steals the Q7 SWDGE SBUF ports) -> PE matmul accumulates the
    SUM-aggregated neighborhood feature-major in PSUM -> fp32r dense
    matmuls (self / neigh in separate PSUM banks) -> bias on self path
    (ACT) -> PE transposes to node-major -> 1/deg scale (ACT
    per-partition) + add + ReLU -> L2 norm + residual -> bf16 cast-DMA
    -> per-half AllGather rebuilds the global feature table.
"""

import math
import sys

import numpy as np
import ml_dtypes

for _p in ("/opt/trn_rl_repo", "/root/.axon_site/_ro/trn_rl_repo"):
    if _p not in sys.path:
        sys.path.append(_p)

import concourse.bacc as bacc  # noqa: E402
import concourse.bass as bass  # noqa: E402
import concourse.mybir as mybir  # noqa: E402
import concourse.tile as tile  # noqa: E402
from concourse import bass_utils  # noqa: E402
from concourse.masks import make_identity  # noqa: E402

M = 8  # cores
D = 128
P = 128
NBUC = 4  # src buckets (int16 index range)
GRP = 4  # dst blocks per dense group
NQ = 4  # SWDGE queues

LAST_EXEC_NS = None  # set by kernel() when _trace=True


def _host_prep(x, src, dst, n_nodes):
    N = n_nodes
    NPC = math.ceil(N / M)
    SLOTS = math.ceil(NPC / P) * P
    NBLK = SLOTS // P
    assert NBLK % 2 == 0
    HBLK = NBLK // 2
    HS = SLOTS // 2
    TBL = M * SLOTS
    BUC = TBL // NBUC
    assert BUC * NBUC == TBL and BUC <= 32768

    x = np.asarray(x).astype(np.int64)
    src = np.asarray(src).astype(np.int64)
    dst = np.asarray(dst).astype(np.int64)

    deg = np.bincount(dst, minlength=N)
    core_of_node = np.minimum(np.arange(N) // NPC, M - 1)
    perm = np.empty(N, np.int64)
    for c in range(M):
        lo, hi = c * NPC, min((c + 1) * NPC, N)
        nodes = np.arange(lo, hi)
        order = np.argsort(deg[nodes], kind="stable")
        r = np.empty(len(nodes), np.int64)
        r[order] = np.arange(len(nodes))
        perm[nodes] = r
    gslot = core_of_node * SLOTS + perm  # output (core, slot) layout
    # half-major global table row: (half, core, slot-within-half)
    grow = (perm // HS) * (M * HS) + core_of_node * HS + (perm % HS)

    ecore = core_of_node[dst]
    # per-core edge arrays
    cores_edges = []
    cnt_cjb = np.zeros((M, NBLK, NBUC), np.int64)
    for c in range(M):
        sel = ecore == c
        dslot = perm[dst[sel]]
        sg = grow[src[sel]]
        buc = sg // BUC
        blk = dslot // P
        o = np.lexsort((dslot, buc, blk))
        dslot, sg, buc, blk = dslot[o], sg[o], buc[o], blk[o]
        cores_edges.append((dslot, sg, buc, blk))
        np.add.at(cnt_cjb[c], (blk, buc), 1)

    # shared per-(block,bucket) edge-slot allocation (max over cores)
    A_jb = cnt_cjb.max(axis=0)  # [NBLK, NBUC]
    for j in range(NBLK):
        if A_jb[j].sum() == 0:
            A_jb[j, 0] = 1  # zero-degree block still produces neigh=0

    # balance blocks into groups of <= GRP within each half
    C_j = A_jb.sum(axis=1)
    groups = []
    nh0 = 0
    for h in range(2):
        blocks = list(range(h * HBLK, (h + 1) * HBLK))
        ngroups = math.ceil(len(blocks) / GRP)
        order = sorted(blocks, key=lambda j: -C_j[j])
        gsum = np.zeros(ngroups, np.int64)
        gcnt = np.zeros(ngroups, np.int64)
        hgroups = [[] for _ in range(ngroups)]
        for j in order:
            cand = [g for g in range(ngroups) if gcnt[g] < GRP]
            g = min(cand, key=lambda q: gsum[q])
            hgroups[g].append(int(j))
            gsum[g] += C_j[j]
            gcnt[g] += 1
        hgroups = [sorted(g) for g in hgroups]
        groups.extend(hgroups)
        if h == 0:
            nh0 = len(hgroups)

    # stream layout: for g, for b: packed segment of the group's blocks
    off_jb = np.zeros((NBLK, NBUC), np.int64)  # edge-slot offset in stream
    calls = []  # per group: list of (b, ch0, nch, ni)
    seg_of = {}  # (j, b) -> (ch0, seg_off)
    pos = 0  # stream position in chunks
    for g in groups:
        gc = []
        for b in range(NBUC):
            seg = 0
            for j in g:
                seg_of[(j, b)] = (pos, seg)
                off_jb[j, b] = pos * P + seg
                seg += int(A_jb[j, b])
            if seg > 0:
                nch = math.ceil(seg / P)
                gc.append((b, pos, nch, seg))
                pos += nch
        calls.append(gc)
    NCH = pos
    NIDX = NCH * P

    # (chunk, block) matmul pairs and one-hot columns, block-major 4-aligned
    blockmm = {j: [] for j in range(NBLK)}  # (b, sp, dc)
    dcol0 = np.zeros(NBLK, np.int64)
    dstcol_jb = np.zeros((NBLK, NBUC), np.int64)  # first dc of (j,b)
    dpos = 0
    for j in range(NBLK):
        dcol0[j] = dpos
        i = 0
        for b in range(NBUC):
            if A_jb[j, b] == 0:
                dstcol_jb[j, b] = -1
                continue
            ch0, so = seg_of[(j, b)]
            lo = ch0 * P + so
            hi = lo + int(A_jb[j, b])
            c_lo, c_hi = lo // P, (hi - 1) // P
            dstcol_jb[j, b] = dpos + i
            for sp in range(c_lo, c_hi + 1):
                blockmm[j].append((b, sp, dpos + i))
                i += 1
        dpos += 4 * math.ceil(i / 4)
    NCHD = dpos

    per_core = []
    for c in range(M):
        dslot, sg, buc, blk = cores_edges[c]
        # rank within (block, bucket)
        flat = (blk * NBUC + buc).astype(np.int64)
        cnts = cnt_cjb[c].reshape(-1)
        st = np.zeros(NBLK * NBUC, np.int64)
        st[1:] = np.cumsum(cnts)[:-1]
        rank = np.arange(len(dslot)) - st[flat]
        spos = off_jb[blk, buc] + rank  # stream row of each edge
        idxs = np.zeros(NIDX, np.int16)
        idxs[spos] = (sg - buc * BUC).astype(np.int16)
        # one-hot column: edge's (chunk, block) pair
        dstloc = np.full((P, NCHD), 255.0, ml_dtypes.bfloat16)
        ech = spos // P  # stream chunk of edge
        # dc = dstcol_jb[j,b] + (ech - first chunk of (j,b) range)
        first_ch = off_jb[blk, buc] // P
        dc = dstcol_jb[blk, buc] + (ech - first_ch)
        dstloc[spos % P, dc] = (dslot % P).astype(np.float32)

        idx16 = idxs.reshape(NIDX // 16, 16).T.copy()  # [16, NIDX/16]
        idx_full = np.tile(idx16, (8, 1))  # [128, NIDX/16]

        lo = c * NPC
        invd = 1.0 / np.maximum(deg, 1.0)
        nodes = np.arange(lo, min((c + 1) * NPC, N))
        node_of_slot = np.full(SLOTS, -1, np.int64)
        node_of_slot[perm[nodes]] = nodes
        invd_slot = np.ones(SLOTS, np.float32)
        real = node_of_slot >= 0
        invd_slot[real] = invd[node_of_slot[real]].astype(np.float32)
        invd_sb = invd_slot.reshape(NBLK, P).T.copy()  # [P, NBLK]

        x_slot = np.zeros(SLOTS, np.int64)
        x_slot[perm[nodes]] = x[nodes]
        xg = x_slot.astype(np.int16)  # emb gather stream (slot order)
        xg16 = np.tile(xg.reshape(SLOTS // 16, 16).T.copy(), (8, 1))

        per_core.append(
            {"gidx": idx_full, "dstloc": dstloc, "invd": invd_sb, "xgidx": xg16}
        )

    meta = {
        "NPC": NPC,
        "SLOTS": SLOTS,
        "NBLK": NBLK,
        "HBLK": HBLK,
        "HS": HS,
        "TBL": TBL,
        "BUC": BUC,
        "groups": groups,
        "nh0": nh0,
        "calls": calls,
        "blockmm": blockmm,
        "dcol0": dcol0,
        "NCH": NCH,
        "NCHD": NCHD,
        "NIDX": NIDX,
        "gslot": gslot,
    }
    return per_core, meta


def _build_program(meta, V, L, single_core=False):
    SLOTS, NBLK, TBL, BUC = meta["SLOTS"], meta["NBLK"], meta["TBL"], meta["BUC"]
    HBLK, HS = meta["HBLK"], meta["HS"]
    groups, nh0, calls, blockmm = (
        meta["groups"],
        meta["nh0"],
        meta["calls"],
        meta["blockmm"],
    )
    dcol0 = meta["dcol0"]
    NCH, NCHD, NIDX = meta["NCH"], meta["NCHD"], meta["NIDX"]
    CBMAX = max(nch for gc in calls for (_, _, nch, _) in gc)

    f32, f32r, bf16 = mybir.dt.float32, mybir.dt.float32r, mybir.dt.bfloat16
    i16 = mybir.dt.int16

    nc = bacc.Bacc(
        "TRN2",
        target_bir_lowering=False,
        debug=False,
        enable_asserts=False,
        num_devices=1 if single_core else M,
        num_swdge_queues=NQ,
    )

    gidx_d = nc.dram_tensor("gidx", [P, NIDX // 16], i16, kind="ExternalInput")
    dstloc_d = nc.dram_tensor("dstloc", [P, NCHD], bf16, kind="ExternalInput")
    invd_d = nc.dram_tensor("invd", [P, NBLK], f32, kind="ExternalInput")
    xgidx_d = nc.dram_tensor("xgidx", [P, SLOTS // 16], i16, kind="ExternalInput")
    emb_d = nc.dram_tensor("emb", [V, D], f32, kind="ExternalInput")
    ws_d = nc.dram_tensor("ws", [L, D, D], f32, kind="ExternalInput")
    wn_d = nc.dram_tensor("wn", [L, D, D], f32, kind="ExternalInput")
    bias_d = nc.dram_tensor("bias", [L, D], f32, kind="ExternalInput")
    hout_d = nc.dram_tensor("hout", [SLOTS, D], f32, kind="ExternalOutput")

    h_shard = nc.dram_tensor("h_shard", [SLOTS, D], bf16, kind="Internal")
    # double-buffered global table: layer l gathers from tab[l%2] while
    # its output AllGathers into tab[(l+1)%2] (no WAR on the live table)
    h_full_t = [
        nc.dram_tensor(
            f"h_full{t}", [TBL, D], bf16, kind="Internal", addr_space="Shared"
        )
        for t in range(2)
    ]

    rg = [list(range(M))]
    qrr = [0]  # gather queue round-robin counter

    with tile.TileContext(nc) as tc:
        with (
            tc.tile_pool(name="const", bufs=1) as cpool,
            tc.tile_pool(name="state", bufs=1) as spool,
            tc.tile_pool(name="gath", bufs=6) as gpool,
            tc.tile_pool(name="oh", bufs=14) as ohpool,
            tc.tile_pool(name="fm", bufs=2) as fmpool,
            tc.tile_pool(name="small", bufs=3) as smpool,
            tc.tile_pool(name="ps_blk", bufs=2, space="PSUM") as ps_blk,
            tc.tile_pool(name="ps_nm", bufs=2, space="PSUM") as ps_nm,
            tc.tile_pool(name="ps_d", bufs=2, space="PSUM") as ps_d,
        ):
            # ---- constants ----
            ident_f = cpool.tile([P, P], f32, tag="ident_f")
            make_identity(nc, ident_f[:])
            # iota repeated 4x along free dim: iota4[p, r*128 + c] = c
            iota4 = cpool.tile([P, 4 * P], bf16, tag="iota4")
            nc.gpsimd.iota(
                iota4[:].rearrange("p (r c) -> p r c", c=P),
                pattern=[[0, 4], [1, P]],
                base=0,
                channel_multiplier=0,
                allow_small_or_imprecise_dtypes=True,
            )

            gidx_sb = cpool.tile([P, NIDX // 16], i16, tag="gidx")
            nc.sync.dma_start(gidx_sb[:], gidx_d[:, :])
            dstloc_sb = cpool.tile([P, NCHD], bf16, tag="dstloc")
            nc.sync.dma_start(dstloc_sb[:], dstloc_d[:, :])
            invd_sb = cpool.tile([P, NBLK], f32, tag="invd")
            nc.sync.dma_start(invd_sb[:], invd_d[:, :])
            xg_sb = cpool.tile([P, SLOTS // 16], i16, tag="xgidx")
            nc.sync.dma_start(xg_sb[:], xgidx_d[:, :])

            w_sb = []
            for l in range(L):
                wsf = cpool.tile([P, D], f32, tag=f"wsf{l}")
                wnf = cpool.tile([P, D], f32, tag=f"wnf{l}")
                nc.sync.dma_start(wsf[:], ws_d[l, :, :])
                nc.sync.dma_start(wnf[:], wn_d[l, :, :])
                ws = cpool.tile([P, D], f32r, tag=f"ws{l}")
                wn = cpool.tile([P, D], f32r, tag=f"wn{l}")
                nc.scalar.copy(ws[:], wsf[:])
                nc.scalar.copy(wn[:], wnf[:])
                w_sb.append((ws, wn))
            b_sb = cpool.tile([P, L], f32, tag="bias")
            for l in range(L):
                nc.sync.dma_start(b_sb[:, l : l + 1], bias_d[l, :, None])

            # ---- embedding lookup: 4 dma_gather calls, one per queue ----
            e_sb = spool.tile([P, NBLK * D], f32, tag="e")
            ev = e_sb[:].rearrange("p (j f) -> p j f", f=D)
            jsplit = [0, 25, 50, 75, NBLK]
            for qi in range(4):
                j0, j1 = jsplit[qi], jsplit[qi + 1]
                ni = (j1 - j0) * P
                nc.gpsimd.dma_gather(
                    ev[:, j0:j1, :],
                    emb_d[:, :],
                    xg_sb[:, j0 * 8 : j1 * 8],
                    ni,
                    ni,
                    D,
                    single_packet=False,
                    queue_num=qi % NQ,
                )

            h_sb = spool.tile([P, NBLK * D], f32, tag="h")

            # zero-init gather ring buffers: ungathered tail rows of a
            # segment's last chunk must not be NaN (NaN * 0 = NaN)
            for _ in range(6):
                gz = gpool.tile([P, CBMAX, D], bf16, tag="gath")
                nc.vector.memset(gz[:, :, :], 0.0)

            shard_v = h_shard.ap().rearrange("(j p) f -> p j f", p=P)

            def store_half(src_tile, h, tab):
                j0, j1 = h * HBLK, (h + 1) * HBLK
                sv = src_tile[:, j0 * D : j1 * D].rearrange(
                    "p (j f) -> p j f", f=D
                )
                nc.gpsimd.dma_start(out=shard_v[:, j0:j1, :], in_=sv)  # cast
                if single_core:
                    return
                nc.gpsimd.collective_compute(
                    "AllGather",
                    mybir.AluOpType.bypass,
                    replica_groups=rg,
                    ins=[h_shard[h * HS : (h + 1) * HS, :]],
                    outs=[tab[h * M * HS : (h + 1) * M * HS, :]],
                )

            store_half(e_sb, 0, h_full_t[0])
            store_half(e_sb, 1, h_full_t[0])

            # ---- layers ----
            for l in range(L):
                cur = e_sb if l == 0 else h_sb
                h_full = h_full_t[l % 2]
                ws, wn = w_sb[l]
                for gi, grp in enumerate(groups):
                    gtiles = {}
                    for (b, ch0, nch, ni) in calls[gi]:
                        gt = gpool.tile([P, CBMAX, D], bf16, tag="gath")
                        nc.gpsimd.dma_gather(
                            gt[:, 0:nch, :],
                            h_full[b * BUC : (b + 1) * BUC, :],
                            gidx_sb[:, ch0 * 8 : (ch0 + nch) * 8],
                            ni,
                            ni,
                            D,
                            single_packet=False,
                            queue_num=qrr[0] % NQ,
                        )
                        qrr[0] += 1
                        gtiles[b] = (gt, ch0)
                    nfm = fmpool.tile([P, GRP * D], f32r, tag="nfm")
                    hfm = fmpool.tile([P, GRP * D], f32r, tag="hfm")
                    for bi, j in enumerate(grp):
                        mms = blockmm[j]
                        nmm = len(mms)
                        # one-hot selectors, 4 columns per DVE op (1-port
                        # mode: never locks GpSimd out of the SBUF ports)
                        nquad = (nmm + 3) // 4
                        d0 = int(dcol0[j])
                        ohqs = []
                        for t in range(nquad):
                            ohq = ohpool.tile([P, 4 * P], bf16, tag="oh")
                            s = d0 + 4 * t
                            nc.vector.tensor_tensor(
                                out=ohq[:].rearrange("p (r c) -> p r c", c=P),
                                in0=dstloc_sb[:, s : s + 4].to_broadcast(
                                    [P, 4, P]
                                ),
                                in1=iota4[:].rearrange("p (r c) -> p r c", c=P),
                                op=mybir.AluOpType.is_equal,
                            )
                            ohqs.append(ohq)
                        blk_ps = ps_blk.tile([P, 2 * P], f32, tag="blk")
                        pa, pt = blk_ps[:, 0:P], blk_ps[:, P : 2 * P]
                        for ci, (b, sp, dc) in enumerate(mms):
                            gt, ch0 = gtiles[b]
                            q, r = (dc - d0) // 4, (dc - d0) % 4
                            nc.tensor.matmul(
                                pa,
                                gt[:, sp - ch0, :],
                                ohqs[q][:, r * P : (r + 1) * P],
                                start=(ci == 0),
                                stop=(ci == nmm - 1),
                            )
                        # pa is feature-major sum-aggregated neigh
                        nc.scalar.copy(nfm[:, bi * D : (bi + 1) * D], pa)
                        nc.tensor.transpose(
                            pt, cur[:, j * D : (j + 1) * D], ident_f[:]
                        )
                        nc.scalar.copy(hfm[:, bi * D : (bi + 1) * D], pt)
                    gw = len(grp) * D
                    d_ps = ps_d.tile([P, 2 * GRP * D], f32, tag="d")
                    pdS = d_ps[:, 0 : GRP * D]
                    pdN = d_ps[:, GRP * D : 2 * GRP * D]
                    nc.tensor.matmul(
                        pdS[:, 0:gw], ws[:], hfm[:, 0:gw], start=True, stop=True
                    )
                    nc.tensor.matmul(
                        pdN[:, 0:gw], wn[:], nfm[:, 0:gw], start=True, stop=True
                    )
                    # self path + bias (feature-major, per-partition bias)
                    hbias = fmpool.tile([P, GRP * D], f32, tag="hbias")
                    nc.scalar.activation(
                        hbias[:, 0:gw],
                        pdS[:, 0:gw],
                        mybir.ActivationFunctionType.Identity,
                        bias=b_sb[:, l : l + 1],
                    )
                    nden = fmpool.tile([P, GRP * D], f32, tag="nden")
                    nc.scalar.copy(nden[:, 0:gw], pdN[:, 0:gw])
                    for bi, j in enumerate(grp):
                        nm_ps = ps_nm.tile([P, 2 * P], f32, tag="nm")
                        pnS, pnN = nm_ps[:, 0:P], nm_ps[:, P : 2 * P]
                        nc.tensor.transpose(
                            pnS, hbias[:, bi * D : (bi + 1) * D], ident_f[:]
                        )
                        nc.tensor.transpose(
                            pnN, nden[:, bi * D : (bi + 1) * D], ident_f[:]
                        )
                        # node-major: neigh * (1/deg), + self, ReLU
                        tn = smpool.tile([P, D], f32, tag="tn")
                        nc.scalar.activation(
                            tn[:],
                            pnN,
                            mybir.ActivationFunctionType.Identity,
                            scale=invd_sb[:, j : j + 1],
                        )
                        hp = smpool.tile([P, D], f32, tag="hp")
                        nc.vector.tensor_tensor(
                            out=hp[:], in0=pnS, in1=tn[:],
                            op=mybir.AluOpType.add,
                        )
                        hr = smpool.tile([P, D], f32, tag="hr")
                        sq_ss = smpool.tile([P, 1], f32, tag="ss")
                        nc.scalar.activation(
                            hr[:], hp[:], mybir.ActivationFunctionType.Relu
                        )
                        sq = smpool.tile([P, D], f32, tag="sq")
                        nc.scalar.activation(
                            sq[:],
                            hr[:],
                            mybir.ActivationFunctionType.Square,
                            accum_out=sq_ss[:],
                        )
                        nrm = smpool.tile([P, 1], f32, tag="nrm")
                        nc.scalar.sqrt(nrm[:], sq_ss[:])
                        nc.vector.tensor_scalar_max(nrm[:], nrm[:], 1e-12)
                        inv = smpool.tile([P, 1], f32, tag="inv")
                        nc.vector.reciprocal(inv[:], nrm[:])
                        htmp = smpool.tile([P, D], f32, tag="htmp")
                        nc.vector.tensor_tensor(
                            out=htmp[:],
                            in0=hr[:],
                            in1=inv[:, 0:1].to_broadcast([P, D]),
                            op=mybir.AluOpType.mult,
                        )
                        nc.vector.tensor_tensor(
                            out=h_sb[:, j * D : (j + 1) * D],
                            in0=htmp[:],
                            in1=e_sb[:, j * D : (j + 1) * D],
                            op=mybir.AluOpType.add,
                        )
                    if l < L - 1 and gi == nh0 - 1:
                        store_half(h_sb, 0, h_full_t[(l + 1) % 2])
                if l < L - 1:
                    store_half(h_sb, 1, h_full_t[(l + 1) % 2])

            hout_v = hout_d.ap().rearrange("(j p) f -> p j f", p=P)
            h_v = h_sb[:].rearrange("p (j f) -> p j f", f=D)
            nc.sync.dma_start(hout_v, h_v)

    nc.compile()
    return nc


def kernel(x, src, dst, emb, Ws, Wn, b, _trace=False):
    x = np.asarray(x)
    src = np.asarray(src)
    dst = np.asarray(dst)
    emb = np.ascontiguousarray(np.asarray(emb, dtype=np.float32))
    Ws = np.ascontiguousarray(np.asarray(Ws, dtype=np.float32))
    Wn = np.ascontiguousarray(np.asarray(Wn, dtype=np.float32))
    b = np.ascontiguousarray(np.asarray(b, dtype=np.float32))
    N = x.shape[0]
    V, _ = emb.shape
    L = Ws.shape[0]

    per_core, meta = _host_prep(x, src, dst, N)
    nc = _build_program(meta, V, L)

    in_maps = []
    for c in range(M):
        pc = per_core[c]
        in_maps.append(
            {
                "gidx": np.ascontiguousarray(pc["gidx"]),
                "dstloc": np.ascontiguousarray(pc["dstloc"]),
                "invd": np.ascontiguousarray(pc["invd"]),
                "xgidx": np.ascontiguousarray(pc["xgidx"]),
                "emb": emb,
                "ws": Ws,
                "wn": Wn,
                "bias": b,
            }
        )

    res = bass_utils.run_bass_kernel_spmd(
        nc, in_maps, core_ids=list(range(M)), trace=_trace
    )
    global LAST_EXEC_NS
    LAST_EXEC_NS = res.exec_time_ns
    outs = [np.asarray(r["hout"], dtype=np.float32) for r in res.results]
    big = np.concatenate(outs, axis=0)
    return big[meta["gslot"]]


# revision 45
# speedup vs baseline: 2.8651x; 1.2537x over previous
"""GraphSAGE (mean) 3-layer encoder on 8 Trainium2 NeuronCores.

Strategy (graph/data parallel, per sharding hint):
  - Nodes sharded contiguously across 8 cores (12500/core, padded to
    12544 = 98*128 "slots"); per-core nodes permuted by in-degree.
  - Global feature table laid out half-major (half, core, slot) so each
    half of a core's shard AllGathers into a contiguous table range;
    the two AllGathers per layer overlap with compute (half-0 launches
    mid-layer, half-1 only blocks next layer's bucket-2/3 gathers).
  - Edges routed by dst core on the host. Per 128-slot dst block, edges
    are grouped by src bucket (4 slices of 25088 table rows so
    dma_gather's int16 indices reach every row) and packed densely into
    per-(group,bucket) segments using shared per-(block,bucket) slot
    allocations (max edge count over cores) -> ~6% padding. Chunks of
    128 gathered rows may straddle adjacent dst blocks; each (chunk,
    block) pair gets its own one-hot column set.
  - dma_gather calls round-robin over 4 SWDGE queues: descriptor
    generation parallelizes across Q7 cores (~4x).
  - Per layer on device: dma_gather of src rows (bf16) -> pure 0/1
    one-hot selectors built 4 columns at a time by ONE DVE
    tensor_tensor (is_equal vs broadcast dst-slot columns; 1-port mode,
    never steals the Q7 SWDGE SBUF ports) -> PE matmul accumulates the
    SUM-aggregated neighborhood feature-major in PSUM -> fp32r dense
    matmuls (self / neigh in separate PSUM banks) -> bias on self path
    (ACT) -> PE transposes to node-major -> 1/deg scale (ACT
    per-partition) + add + ReLU -> L2 norm + residual -> bf16 cast-DMA
    -> per-half AllGather rebuilds the global feature table.
"""

import math
import sys

import numpy as np
import ml_dtypes

for _p in ("/opt/trn_rl_repo", "/root/.axon_site/_ro/trn_rl_repo"):
    if _p not in sys.path:
        sys.path.append(_p)

import concourse.bacc as bacc  # noqa: E402
import concourse.bass as bass  # noqa: E402
import concourse.mybir as mybir  # noqa: E402
import concourse.tile as tile  # noqa: E402
from concourse import bass_utils  # noqa: E402
from concourse.masks import make_identity  # noqa: E402

M = 8  # cores
D = 128
P = 128
NBUC = 4  # src buckets (int16 index range)
GRP = 4  # dst blocks per dense group
NQ = 4  # SWDGE queues
VT = 40  # vocab tiles (layer-0 count-matrix aggregation)
VP = VT * P  # padded vocab
EMB_SCALE = 64.0  # emb upscale for fp8 (undone via layer-0 1/deg scale)

LAST_EXEC_NS = None  # set by kernel() when _trace=True


def _host_prep(x, src, dst, n_nodes):
    N = n_nodes
    NPC = math.ceil(N / M)
    SLOTS = math.ceil(NPC / P) * P
    NBLK = SLOTS // P
    assert NBLK % 2 == 0
    HBLK = NBLK // 2
    HS = SLOTS // 2
    TBL = M * SLOTS
    BUC = TBL // NBUC
    assert BUC * NBUC == TBL and BUC <= 32768

    x = np.asarray(x).astype(np.int64)
    src = np.asarray(src).astype(np.int64)
    dst = np.asarray(dst).astype(np.int64)

    deg = np.bincount(dst, minlength=N)
    core_of_node = np.minimum(np.arange(N) // NPC, M - 1)
    perm = np.empty(N, np.int64)
    for c in range(M):
        lo, hi = c * NPC, min((c + 1) * NPC, N)
        nodes = np.arange(lo, hi)
        order = np.argsort(deg[nodes], kind="stable")
        r = np.empty(len(nodes), np.int64)
        r[order] = np.arange(len(nodes))
        perm[nodes] = r
    gslot = core_of_node * SLOTS + perm  # output (core, slot) layout
    # half-major global table row: (half, core, slot-within-half)
    grow = (perm // HS) * (M * HS) + core_of_node * HS + (perm % HS)

    ecore = core_of_node[dst]
    # per-core edge arrays
    cores_edges = []
    cnt_cjb = np.zeros((M, NBLK, NBUC), np.int64)
    for c in range(M):
        sel = ecore == c
        dslot = perm[dst[sel]]
        sg = grow[src[sel]]
        buc = sg // BUC
        blk = dslot // P
        o = np.lexsort((dslot, buc, blk))
        dslot, sg, buc, blk = dslot[o], sg[o], buc[o], blk[o]
        cores_edges.append((dslot, sg, buc, blk))
        np.add.at(cnt_cjb[c], (blk, buc), 1)

    # shared per-(block,bucket) edge-slot allocation (max over cores)
    A_jb = cnt_cjb.max(axis=0)  # [NBLK, NBUC]
    for j in range(NBLK):
        if A_jb[j].sum() == 0:
            A_jb[j, 0] = 1  # zero-degree block still produces neigh=0

    # balance blocks into groups of <= GRP within each half
    C_j = A_jb.sum(axis=1)
    groups = []
    nh0 = 0
    for h in range(2):
        blocks = list(range(h * HBLK, (h + 1) * HBLK))
        ngroups = math.ceil(len(blocks) / GRP)
        order = sorted(blocks, key=lambda j: -C_j[j])
        gsum = np.zeros(ngroups, np.int64)
        gcnt = np.zeros(ngroups, np.int64)
        hgroups = [[] for _ in range(ngroups)]
        for j in order:
            cand = [g for g in range(ngroups) if gcnt[g] < GRP]
            g = min(cand, key=lambda q: gsum[q])
            hgroups[g].append(int(j))
            gsum[g] += C_j[j]
            gcnt[g] += 1
        hgroups = [sorted(g) for g in hgroups]
        groups.extend(hgroups)
        if h == 0:
            nh0 = len(hgroups)

    # stream layout: for g, for b: packed segment of the group's blocks
    off_jb = np.zeros((NBLK, NBUC), np.int64)  # edge-slot offset in stream
    calls = []  # per group: list of (b, ch0, nch, ni)
    seg_of = {}  # (j, b) -> (ch0, seg_off)
    pos = 0  # stream position in chunks
    for g in groups:
        gc = []
        for b in range(NBUC):
            seg = 0
            for j in g:
                seg_of[(j, b)] = (pos, seg)
                off_jb[j, b] = pos * P + seg
                seg += int(A_jb[j, b])
            if seg > 0:
                nch = math.ceil(seg / P)
                gc.append((b, pos, nch, seg))
                pos += nch
        calls.append(gc)
    NCH = pos
    NIDX = NCH * P

    # (chunk, block) matmul pairs and one-hot columns, block-major 4-aligned
    blockmm = {j: [] for j in range(NBLK)}  # (b, sp, dc)
    dcol0 = np.zeros(NBLK, np.int64)
    dstcol_jb = np.zeros((NBLK, NBUC), np.int64)  # first dc of (j,b)
    dpos = 0
    for j in range(NBLK):
        dcol0[j] = dpos
        i = 0
        for b in range(NBUC):
            if A_jb[j, b] == 0:
                dstcol_jb[j, b] = -1
                continue
            ch0, so = seg_of[(j, b)]
            lo = ch0 * P + so
            hi = lo + int(A_jb[j, b])
            c_lo, c_hi = lo // P, (hi - 1) // P
            dstcol_jb[j, b] = dpos + i
            for sp in range(c_lo, c_hi + 1):
                blockmm[j].append((b, sp, dpos + i))
                i += 1
        dpos += 8 * math.ceil(i / 8)
    NCHD = dpos

    per_core = []
    for c in range(M):
        dslot, sg, buc, blk = cores_edges[c]
        # layer-0 count matrix (vocab -> dst slot), SBUF-layout tiled:
        # C8s[j, p, t*P + s] = #edges with x[src] == t*P+p, dst slot j*P+s
        sel = ecore == c
        cnt = np.zeros((VP, SLOTS), np.int16)
        np.add.at(cnt, (x[src[sel]], perm[dst[sel]]), 1)
        c8 = (
            cnt.reshape(VT, P, NBLK, P)
            .transpose(2, 1, 0, 3)
            .reshape(NBLK, P, VT * P)
            .astype(ml_dtypes.float8_e4m3fn)
        )
        # rank within (block, bucket)
        flat = (blk * NBUC + buc).astype(np.int64)
        cnts = cnt_cjb[c].reshape(-1)
        st = np.zeros(NBLK * NBUC, np.int64)
        st[1:] = np.cumsum(cnts)[:-1]
        rank = np.arange(len(dslot)) - st[flat]
        spos = off_jb[blk, buc] + rank  # stream row of each edge
        idxs = np.zeros(NIDX, np.int16)
        idxs[spos] = (sg - buc * BUC).astype(np.int16)
        # one-hot column: edge's (chunk, block) pair
        dstloc = np.full((P, NCHD), 255.0, ml_dtypes.bfloat16)
        ech = spos // P  # stream chunk of edge
        # dc = dstcol_jb[j,b] + (ech - first chunk of (j,b) range)
        first_ch = off_jb[blk, buc] // P
        dc = dstcol_jb[blk, buc] + (ech - first_ch)
        dstloc[spos % P, dc] = (dslot % P).astype(np.float32)

        idx16 = idxs.reshape(NIDX // 16, 16).T.copy()  # [16, NIDX/16]
        idx_full = np.tile(idx16, (8, 1))  # [128, NIDX/16]

        lo = c * NPC
        invd = 1.0 / np.maximum(deg, 1.0)
        nodes = np.arange(lo, min((c + 1) * NPC, N))
        node_of_slot = np.full(SLOTS, -1, np.int64)
        node_of_slot[perm[nodes]] = nodes
        invd_slot = np.ones(SLOTS, np.float32)
        real = node_of_slot >= 0
        invd_slot[real] = invd[node_of_slot[real]].astype(np.float32)
        invd_sb = invd_slot.reshape(NBLK, P).T.copy()  # [P, NBLK]

        x_slot = np.zeros(SLOTS, np.int64)
        x_slot[perm[nodes]] = x[nodes]
        xg = x_slot.astype(np.int16)  # emb gather stream (slot order)
        xg16 = np.tile(xg.reshape(SLOTS // 16, 16).T.copy(), (8, 1))

        per_core.append(
            {
                "gidx": idx_full,
                "dstloc": dstloc,
                "invd": invd_sb,
                "invd0": invd_sb / EMB_SCALE,
                "xgidx": xg16,
                "c8": c8,
            }
        )

    meta = {
        "NPC": NPC,
        "SLOTS": SLOTS,
        "NBLK": NBLK,
        "HBLK": HBLK,
        "HS": HS,
        "TBL": TBL,
        "BUC": BUC,
        "groups": groups,
        "nh0": nh0,
        "calls": calls,
        "blockmm": blockmm,
        "dcol0": dcol0,
        "NCH": NCH,
        "NCHD": NCHD,
        "NIDX": NIDX,
        "gslot": gslot,
    }
    return per_core, meta


def _build_program(meta, V, L, single_core=False):
    SLOTS, NBLK, TBL, BUC = meta["SLOTS"], meta["NBLK"], meta["TBL"], meta["BUC"]
    HBLK, HS = meta["HBLK"], meta["HS"]
    groups, nh0, calls, blockmm = (
        meta["groups"],
        meta["nh0"],
        meta["calls"],
        meta["blockmm"],
    )
    dcol0 = meta["dcol0"]
    NCH, NCHD, NIDX = meta["NCH"], meta["NCHD"], meta["NIDX"]
    CBMAX = max(nch for gc in calls for (_, _, nch, _) in gc)

    f32, f32r, bf16 = mybir.dt.float32, mybir.dt.float32r, mybir.dt.bfloat16
    i16, f8 = mybir.dt.int16, mybir.dt.float8e4

    nc = bacc.Bacc(
        "TRN2",
        target_bir_lowering=False,
        debug=False,
        enable_asserts=False,
        num_devices=1 if single_core else M,
        num_swdge_queues=NQ,
    )

    gidx_d = nc.dram_tensor("gidx", [P, NIDX // 16], i16, kind="ExternalInput")
    dstloc_d = nc.dram_tensor("dstloc", [P, NCHD], bf16, kind="ExternalInput")
    invd_d = nc.dram_tensor("invd", [P, NBLK], f32, kind="ExternalInput")
    invd0_d = nc.dram_tensor("invd0", [P, NBLK], f32, kind="ExternalInput")
    emb8_d = nc.dram_tensor("emb8", [P, VT * D], f8, kind="ExternalInput")
    c8_d = nc.dram_tensor("c8", [NBLK, P, VT * P], f8, kind="ExternalInput")
    xgidx_d = nc.dram_tensor("xgidx", [P, SLOTS // 16], i16, kind="ExternalInput")
    emb16_d = nc.dram_tensor("emb16", [V, D], bf16, kind="ExternalInput")
    ws_d = nc.dram_tensor("ws", [L, D, D], f32, kind="ExternalInput")
    wn_d = nc.dram_tensor("wn", [L, D, D], f32, kind="ExternalInput")
    bias_d = nc.dram_tensor("bias", [L, D], f32, kind="ExternalInput")
    hout_d = nc.dram_tensor("hout", [SLOTS, D], f32, kind="ExternalOutput")

    h_shard = nc.dram_tensor("h_shard", [SLOTS, D], bf16, kind="Internal")
    # double-buffered global table: layer l gathers from tab[l%2] while
    # its output AllGathers into tab[(l+1)%2] (no WAR on the live table)
    h_full_t = [
        nc.dram_tensor(
            f"h_full{t}", [TBL, D], bf16, kind="Internal", addr_space="Shared"
        )
        for t in range(2)
    ]

    rg = [list(range(M))]
    qrr = [0]  # gather queue round-robin counter

    with tile.TileContext(nc) as tc:
        with (
            tc.tile_pool(name="const", bufs=1) as cpool,
            tc.tile_pool(name="state", bufs=1) as spool,
            tc.tile_pool(name="gath", bufs=6) as gpool,
            tc.tile_pool(name="oh", bufs=8) as ohpool,
            tc.tile_pool(name="c8", bufs=2) as c8pool,
            tc.tile_pool(name="fm", bufs=2) as fmpool,
            tc.tile_pool(name="small", bufs=3) as smpool,
            tc.tile_pool(name="ps_blk", bufs=2, space="PSUM") as ps_blk,
            tc.tile_pool(name="ps_nm", bufs=2, space="PSUM") as ps_nm,
            tc.tile_pool(name="ps_d", bufs=2, space="PSUM") as ps_d,
        ):
            # ---- constants ----
            ident_f = cpool.tile([P, P], f32, tag="ident_f")
            make_identity(nc, ident_f[:])

            # iota repeated 8x along free dim: iota8[p, r*128 + c] = c
            iota8 = cpool.tile([P, 8 * P], bf16, tag="iota8")
            nc.gpsimd.iota(
                iota8[:].rearrange("p (r c) -> p r c", c=P),
                pattern=[[0, 8], [1, P]],
                base=0,
                channel_multiplier=0,
                allow_small_or_imprecise_dtypes=True,
            )

            gidx_sb = cpool.tile([P, NIDX // 16], i16, tag="gidx")
            nc.sync.dma_start(gidx_sb[:], gidx_d[:, :])
            dstloc_sb = cpool.tile([P, NCHD], bf16, tag="dstloc")
            nc.sync.dma_start(dstloc_sb[:], dstloc_d[:, :])
            invd_sb = cpool.tile([P, NBLK], f32, tag="invd")
            nc.sync.dma_start(invd_sb[:], invd_d[:, :])
            invd0_sb = cpool.tile([P, NBLK], f32, tag="invd0")
            nc.sync.dma_start(invd0_sb[:], invd0_d[:, :])
            emb8_sb = cpool.tile([P, VT * D], f8, tag="emb8")
            nc.sync.dma_start(emb8_sb[:], emb8_d[:, :])
            xg_sb = cpool.tile([P, SLOTS // 16], i16, tag="xgidx")
            nc.sync.dma_start(xg_sb[:], xgidx_d[:, :])

            w_sb = []
            for l in range(L):
                wsf = cpool.tile([P, D], f32, tag=f"wsf{l}")
                wnf = cpool.tile([P, D], f32, tag=f"wnf{l}")
                nc.sync.dma_start(wsf[:], ws_d[l, :, :])
                nc.sync.dma_start(wnf[:], wn_d[l, :, :])
                ws = cpool.tile([P, D], f32r, tag=f"ws{l}")
                wn = cpool.tile([P, D], f32r, tag=f"wn{l}")
                nc.scalar.copy(ws[:], wsf[:])
                nc.scalar.copy(wn[:], wnf[:])
                w_sb.append((ws, wn))
            b_sb = cpool.tile([P, L], f32, tag="bias")
            for l in range(L):
                nc.sync.dma_start(b_sb[:, l : l + 1], bias_d[l, :, None])

            # ---- embedding lookup: 4 dma_gather calls, one per queue ----
            e_sb = spool.tile([P, NBLK * D], bf16, tag="e")
            ev = e_sb[:].rearrange("p (j f) -> p j f", f=D)
            jsplit = [0, 25, 50, 75, NBLK]
            for qi in range(4):
                j0, j1 = jsplit[qi], jsplit[qi + 1]
                ni = (j1 - j0) * P
                nc.gpsimd.dma_gather(
                    ev[:, j0:j1, :],
                    emb16_d[:, :],
                    xg_sb[:, j0 * 8 : j1 * 8],
                    ni,
                    ni,
                    D,
                    single_packet=False,
                    queue_num=qi % NQ,
                )

            h_sb = spool.tile([P, NBLK * D], f32, tag="h")

            # zero-init gather ring buffers: ungathered tail rows of a
            # segment's last chunk must not be NaN (NaN * 0 = NaN)
            for _ in range(6):
                gz = gpool.tile([P, CBMAX, D], bf16, tag="gath")
                nc.vector.memset(gz[:, :, :], 0.0)

            shard_v = h_shard.ap().rearrange("(j p) f -> p j f", p=P)

            def store_half(src_tile, h, tab):
                j0, j1 = h * HBLK, (h + 1) * HBLK
                sv = src_tile[:, j0 * D : j1 * D].rearrange(
                    "p (j f) -> p j f", f=D
                )
                nc.gpsimd.dma_start(out=shard_v[:, j0:j1, :], in_=sv)  # cast
                if single_core:
                    return
                nc.gpsimd.collective_compute(
                    "AllGather",
                    mybir.AluOpType.bypass,
                    replica_groups=rg,
                    ins=[h_shard[h * HS : (h + 1) * HS, :]],
                    outs=[tab[h * M * HS : (h + 1) * M * HS, :]],
                )

            # no initial e-table AllGather: layer 0 aggregates from the
            # vocab count-matrices, so nothing reads a table until layer 1

            # ---- layers ----
            for l in range(L):
                cur = e_sb if l == 0 else h_sb
                h_full = h_full_t[l % 2]
                ws, wn = w_sb[l]
                for gi, grp in enumerate(groups):
                    if l > 0:
                        gtiles = {}
                        for (b, ch0, nch, ni) in calls[gi]:
                            gt = gpool.tile([P, CBMAX, D], bf16, tag="gath")
                            nc.gpsimd.dma_gather(
                                gt[:, 0:nch, :],
                                h_full[b * BUC : (b + 1) * BUC, :],
                                gidx_sb[:, ch0 * 8 : (ch0 + nch) * 8],
                                ni,
                                ni,
                                D,
                                single_packet=False,
                                queue_num=qrr[0] % NQ,
                            )
                            qrr[0] += 1
                            gtiles[b] = (gt, ch0)
                    nfm = fmpool.tile([P, GRP * D], f32r, tag="nfm")
                    hfm = fmpool.tile([P, GRP * D], f32r, tag="hfm")
                    for bi, j in enumerate(grp):
                        blk_ps = ps_blk.tile([P, 2 * P], f32, tag="blk")
                        pa, pt = blk_ps[:, 0:P], blk_ps[:, P : 2 * P]
                        if l == 0:
                            # layer 0: neigh-sum = emb8^T @ count-matrix
                            # (no gathers: sources live in the 5k vocab)
                            cb = c8pool.tile([P, VT * P], f8, tag="c8")
                            nc.sync.dma_start(cb[:], c8_d[j, :, :])
                            for t in range(VT):
                                nc.tensor.matmul(
                                    pa,
                                    emb8_sb[:, t * D : (t + 1) * D],
                                    cb[:, t * P : (t + 1) * P],
                                    start=(t == 0),
                                    stop=(t == VT - 1),
                                )
                        else:
                            mms = blockmm[j]
                            nmm = len(mms)
                            # one-hot selectors, 8 columns per DVE op
                            # (1-port mode: never locks GpSimd out of
                            # the SBUF ports)
                            noct = (nmm + 7) // 8
                            d0 = int(dcol0[j])
                            ohqs = []
                            for t in range(noct):
                                ohq = ohpool.tile([P, 8 * P], bf16, tag="oh")
                                s = d0 + 8 * t
                                nc.vector.tensor_tensor(
                                    out=ohq[:].rearrange(
                                        "p (r c) -> p r c", c=P
                                    ),
                                    in0=dstloc_sb[:, s : s + 8].to_broadcast(
                                        [P, 8, P]
                                    ),
                                    in1=iota8[:].rearrange(
                                        "p (r c) -> p r c", c=P
                                    ),
                                    op=mybir.AluOpType.is_equal,
                                )
                                ohqs.append(ohq)
                            for ci, (b, sp, dc) in enumerate(mms):
                                gt, ch0 = gtiles[b]
                                q, r = (dc - d0) // 8, (dc - d0) % 8
                                nc.tensor.matmul(
                                    pa,
                                    gt[:, sp - ch0, :],
                                    ohqs[q][:, r * P : (r + 1) * P],
                                    start=(ci == 0),
                                    stop=(ci == nmm - 1),
                                )
                        # pa is feature-major sum-aggregated neigh
                        nc.scalar.copy(nfm[:, bi * D : (bi + 1) * D], pa)
                        if l == 0:
                            cur_f = smpool.tile([P, D], f32, tag="curf")
                            nc.scalar.copy(cur_f[:], cur[:, j * D : (j + 1) * D])
                            nc.tensor.transpose(pt, cur_f[:], ident_f[:])
                        else:
                            nc.tensor.transpose(
                                pt, cur[:, j * D : (j + 1) * D], ident_f[:]
                            )
                        nc.scalar.copy(hfm[:, bi * D : (bi + 1) * D], pt)
                    gw = len(grp) * D
                    d_ps = ps_d.tile([P, 2 * GRP * D], f32, tag="d")
                    pdS = d_ps[:, 0 : GRP * D]
                    pdN = d_ps[:, GRP * D : 2 * GRP * D]
                    nc.tensor.matmul(
                        pdS[:, 0:gw], ws[:], hfm[:, 0:gw], start=True, stop=True
                    )
                    nc.tensor.matmul(
                        pdN[:, 0:gw], wn[:], nfm[:, 0:gw], start=True, stop=True
                    )
                    # self path + bias (feature-major, per-partition bias)
                    hbias = fmpool.tile([P, GRP * D], f32, tag="hbias")
                    nc.scalar.activation(
                        hbias[:, 0:gw],
                        pdS[:, 0:gw],
                        mybir.ActivationFunctionType.Identity,
                        bias=b_sb[:, l : l + 1],
                    )
                    nden = fmpool.tile([P, GRP * D], f32, tag="nden")
                    nc.scalar.copy(nden[:, 0:gw], pdN[:, 0:gw])
                    for bi, j in enumerate(grp):
                        nm_ps = ps_nm.tile([P, 2 * P], f32, tag="nm")
                        pnS, pnN = nm_ps[:, 0:P], nm_ps[:, P : 2 * P]
                        nc.tensor.transpose(
                            pnS, hbias[:, bi * D : (bi + 1) * D], ident_f[:]
                        )
                        nc.tensor.transpose(
                            pnN, nden[:, bi * D : (bi + 1) * D], ident_f[:]
                        )
                        # node-major: neigh * (1/deg), + self, ReLU
                        inv_l = invd0_sb if l == 0 else invd_sb
                        tn = smpool.tile([P, D], f32, tag="tn")
                        nc.scalar.activation(
                            tn[:],
                            pnN,
                            mybir.ActivationFunctionType.Identity,
                            scale=inv_l[:, j : j + 1],
                        )
                        hp = smpool.tile([P, D], f32, tag="hp")
                        nc.vector.tensor_tensor(
                            out=hp[:], in0=pnS, in1=tn[:],
                            op=mybir.AluOpType.add,
                        )
                        hr = smpool.tile([P, D], f32, tag="hr")
                        sq_ss = smpool.tile([P, 1], f32, tag="ss")
                        nc.scalar.activation(
                            hr[:], hp[:], mybir.ActivationFunctionType.Relu
                        )
                        sq = smpool.tile([P, D], f32, tag="sq")
                        nc.scalar.activation(
                            sq[:],
                            hr[:],
                            mybir.ActivationFunctionType.Square,
                            accum_out=sq_ss[:],
                        )
                        nrm = smpool.tile([P, 1], f32, tag="nrm")
                        nc.scalar.sqrt(nrm[:], sq_ss[:])
                        nc.vector.tensor_scalar_max(nrm[:], nrm[:], 1e-12)
                        inv = smpool.tile([P, 1], f32, tag="inv")
                        nc.vector.reciprocal(inv[:], nrm[:])
                        htmp = smpool.tile([P, D], f32, tag="htmp")
                        nc.vector.tensor_tensor(
                            out=htmp[:],
                            in0=hr[:],
                            in1=inv[:, 0:1].to_broadcast([P, D]),
                            op=mybir.AluOpType.mult,
                        )
                        nc.vector.tensor_tensor(
                            out=h_sb[:, j * D : (j + 1) * D],
                            in0=htmp[:],
                            in1=e_sb[:, j * D : (j + 1) * D],
                            op=mybir.AluOpType.add,
                        )
                    if l < L - 1 and gi == nh0 - 1:
                        store_half(h_sb, 0, h_full_t[(l + 1) % 2])
                if l < L - 1:
                    store_half(h_sb, 1, h_full_t[(l + 1) % 2])

            hout_v = hout_d.ap().rearrange("(j p) f -> p j f", p=P)
            h_v = h_sb[:].rearrange("p (j f) -> p j f", f=D)
            nc.sync.dma_start(hout_v, h_v)

    nc.compile()
    return nc


def kernel(x, src, dst, emb, Ws, Wn, b, _trace=False):
    x = np.asarray(x)
    src = np.asarray(src)
    dst = np.asarray(dst)
    emb = np.ascontiguousarray(np.asarray(emb, dtype=np.float32))
    Ws = np.ascontiguousarray(np.asarray(Ws, dtype=np.float32))
    Wn = np.ascontiguousarray(np.asarray(Wn, dtype=np.float32))
    b = np.ascontiguousarray(np.asarray(b, dtype=np.float32))
    N = x.shape[0]
    V, _ = emb.shape
    L = Ws.shape[0]

    per_core, meta = _host_prep(x, src, dst, N)
    nc = _build_program(meta, V, L)

    # emb, upscaled for fp8 and pre-swizzled to the SBUF tile layout:
    # emb8s[p, t*D + f] = (emb * EMB_SCALE)[t*P + p, f]
    embp = np.zeros((VP, D), np.float32)
    embp[:V] = emb * EMB_SCALE
    emb8s = np.ascontiguousarray(
        embp.reshape(VT, P, D).transpose(1, 0, 2).reshape(P, VT * D)
    ).astype(ml_dtypes.float8_e4m3fn)
    emb16 = np.ascontiguousarray(emb.astype(ml_dtypes.bfloat16))

    in_maps = []
    for c in range(M):
        pc = per_core[c]
        in_maps.append(
            {
                "gidx": np.ascontiguousarray(pc["gidx"]),
                "dstloc": np.ascontiguousarray(pc["dstloc"]),
                "invd": np.ascontiguousarray(pc["invd"]),
                "invd0": np.ascontiguousarray(pc["invd0"]),
                "xgidx": np.ascontiguousarray(pc["xgidx"]),
                "c8": np.ascontiguousarray(pc["c8"]),
                "emb8": emb8s,
                "emb16": emb16,
                "ws": Ws,
                "wn": Wn,
                "bias": b,
            }
        )

    res = bass_utils.run_bass_kernel_spmd(
        nc, in_maps, core_ids=list(range(M)), trace=_trace
    )
    global LAST_EXEC_NS
    LAST_EXEC_NS = res.exec_time_ns
    outs = [np.asarray(r["hout"], dtype=np.float32) for r in res.results]
    big = np.concatenate(outs, axis=0)
    return big[meta["gslot"]]


# revision 52
# speedup vs baseline: 2.9066x; 1.0145x over previous
"""GraphSAGE (mean) 3-layer encoder on 8 Trainium2 NeuronCores.

Strategy (graph/data parallel, per sharding hint):
  - Nodes sharded contiguously across 8 cores (12500/core, padded to
    12544 = 98*128 "slots"); per-core nodes permuted by in-degree.
  - Global feature table laid out half-major (half, core, slot) so each
    half of a core's shard AllGathers into a contiguous table range;
    the two AllGathers per layer overlap with compute (half-0 launches
    mid-layer, half-1 only blocks next layer's bucket-2/3 gathers).
  - Edges routed by dst core on the host. Per 128-slot dst block, edges
    are grouped by src bucket (4 slices of 25088 table rows so
    dma_gather's int16 indices reach every row) and packed densely into
    per-(group,bucket) segments using shared per-(block,bucket) slot
    allocations (max edge count over cores) -> ~6% padding. Chunks of
    128 gathered rows may straddle adjacent dst blocks; each (chunk,
    block) pair gets its own one-hot column set.
  - dma_gather calls round-robin over 4 SWDGE queues: descriptor
    generation parallelizes across Q7 cores (~4x).
  - Per layer on device: dma_gather of src rows (bf16) -> pure 0/1
    one-hot selectors built 4 columns at a time by ONE DVE
    tensor_tensor (is_equal vs broadcast dst-slot columns; 1-port mode,
    never steals the Q7 SWDGE SBUF ports) -> PE matmul accumulates the
    SUM-aggregated neighborhood feature-major in PSUM -> fp32r dense
    matmuls (self / neigh in separate PSUM banks) -> bias on self path
    (ACT) -> PE transposes to node-major -> 1/deg scale (ACT
    per-partition) + add + ReLU -> L2 norm + residual -> bf16 cast-DMA
    -> per-half AllGather rebuilds the global feature table.
"""

import math
import sys

import numpy as np
import ml_dtypes

for _p in ("/opt/trn_rl_repo", "/root/.axon_site/_ro/trn_rl_repo"):
    if _p not in sys.path:
        sys.path.append(_p)

import concourse.bacc as bacc  # noqa: E402
import concourse.bass as bass  # noqa: E402
import concourse.mybir as mybir  # noqa: E402
import concourse.tile as tile  # noqa: E402
from concourse import bass_utils  # noqa: E402
from concourse.masks import make_identity  # noqa: E402

M = 8  # cores
D = 128
P = 128
NBUC = 4  # src buckets (int16 index range)
GRP = 4  # dst blocks per dense group
NQ = 4  # SWDGE queues
VT = 40  # vocab tiles (layer-0 count-matrix aggregation)
VP = VT * P  # padded vocab
EMB_SCALE = 64.0  # emb upscale for fp8 (undone via layer-0 1/deg scale)

LAST_EXEC_NS = None  # set by kernel() when _trace=True


def _host_prep(x, src, dst, n_nodes):
    N = n_nodes
    NPC = math.ceil(N / M)
    SLOTS = math.ceil(NPC / P) * P
    NBLK = SLOTS // P
    assert NBLK % 2 == 0
    HBLK = NBLK // 2
    HS = SLOTS // 2
    TBL = M * SLOTS
    BUC = TBL // NBUC
    assert BUC * NBUC == TBL and BUC <= 32768

    x = np.asarray(x).astype(np.int64)
    src = np.asarray(src).astype(np.int64)
    dst = np.asarray(dst).astype(np.int64)

    deg = np.bincount(dst, minlength=N)
    core_of_node = np.minimum(np.arange(N) // NPC, M - 1)
    perm = np.empty(N, np.int64)
    for c in range(M):
        lo, hi = c * NPC, min((c + 1) * NPC, N)
        nodes = np.arange(lo, hi)
        order = np.argsort(deg[nodes], kind="stable")
        r = np.empty(len(nodes), np.int64)
        r[order] = np.arange(len(nodes))
        perm[nodes] = r
    gslot = core_of_node * SLOTS + perm  # output (core, slot) layout
    # half-major global table row: (half, core, slot-within-half)
    grow = (perm // HS) * (M * HS) + core_of_node * HS + (perm % HS)

    ecore = core_of_node[dst]
    # per-core edge arrays
    cores_edges = []
    cnt_cjb = np.zeros((M, NBLK, NBUC), np.int64)
    for c in range(M):
        sel = ecore == c
        dslot = perm[dst[sel]]
        sg = grow[src[sel]]
        buc = sg // BUC
        blk = dslot // P
        o = np.lexsort((dslot, buc, blk))
        dslot, sg, buc, blk = dslot[o], sg[o], buc[o], blk[o]
        cores_edges.append((dslot, sg, buc, blk))
        np.add.at(cnt_cjb[c], (blk, buc), 1)

    # shared per-(block,bucket) edge-slot allocation (max over cores)
    A_jb = cnt_cjb.max(axis=0)  # [NBLK, NBUC]
    for j in range(NBLK):
        if A_jb[j].sum() == 0:
            A_jb[j, 0] = 1  # zero-degree block still produces neigh=0

    # balance blocks into groups of <= GRP within each half
    C_j = A_jb.sum(axis=1)
    groups = []
    nh0 = 0
    for h in range(2):
        blocks = list(range(h * HBLK, (h + 1) * HBLK))
        ngroups = math.ceil(len(blocks) / GRP)
        order = sorted(blocks, key=lambda j: -C_j[j])
        gsum = np.zeros(ngroups, np.int64)
        gcnt = np.zeros(ngroups, np.int64)
        hgroups = [[] for _ in range(ngroups)]
        for j in order:
            cand = [g for g in range(ngroups) if gcnt[g] < GRP]
            g = min(cand, key=lambda q: gsum[q])
            hgroups[g].append(int(j))
            gsum[g] += C_j[j]
            gcnt[g] += 1
        hgroups = [sorted(g) for g in hgroups]
        groups.extend(hgroups)
        if h == 0:
            nh0 = len(hgroups)

    # stream layout: for g, for b: packed segment of the group's blocks
    off_jb = np.zeros((NBLK, NBUC), np.int64)  # edge-slot offset in stream
    calls = []  # per group: list of (b, ch0, nch, ni)
    seg_of = {}  # (j, b) -> (ch0, seg_off)
    pos = 0  # stream position in chunks
    for g in groups:
        gc = []
        for b in range(NBUC):
            seg = 0
            for j in g:
                seg_of[(j, b)] = (pos, seg)
                off_jb[j, b] = pos * P + seg
                seg += int(A_jb[j, b])
            if seg > 0:
                nch = math.ceil(seg / P)
                gc.append((b, pos, nch, seg))
                pos += nch
        calls.append(gc)
    NCH = pos
    NIDX = NCH * P

    # (chunk, block) matmul pairs and one-hot columns, block-major 4-aligned
    blockmm = {j: [] for j in range(NBLK)}  # (b, sp, dc)
    dcol0 = np.zeros(NBLK, np.int64)
    dstcol_jb = np.zeros((NBLK, NBUC), np.int64)  # first dc of (j,b)
    dpos = 0
    for j in range(NBLK):
        dcol0[j] = dpos
        i = 0
        for b in range(NBUC):
            if A_jb[j, b] == 0:
                dstcol_jb[j, b] = -1
                continue
            ch0, so = seg_of[(j, b)]
            lo = ch0 * P + so
            hi = lo + int(A_jb[j, b])
            c_lo, c_hi = lo // P, (hi - 1) // P
            dstcol_jb[j, b] = dpos + i
            for sp in range(c_lo, c_hi + 1):
                blockmm[j].append((b, sp, dpos + i))
                i += 1
        dpos += 8 * math.ceil(i / 8)
    NCHD = dpos

    per_core = []
    for c in range(M):
        dslot, sg, buc, blk = cores_edges[c]
        # layer-0 count matrix (vocab -> dst slot), SBUF-layout tiled:
        # C8s[j, p, t*P + s] = #edges with x[src] == t*P+p, dst slot j*P+s
        sel = ecore == c
        cnt = np.zeros((VP, SLOTS), np.int16)
        np.add.at(cnt, (x[src[sel]], perm[dst[sel]]), 1)
        c8 = (
            cnt.reshape(VT, P, NBLK, P)
            .transpose(2, 1, 0, 3)
            .reshape(NBLK, P, VT * P)
            .astype(ml_dtypes.float8_e4m3fn)
        )
        # rank within (block, bucket)
        flat = (blk * NBUC + buc).astype(np.int64)
        cnts = cnt_cjb[c].reshape(-1)
        st = np.zeros(NBLK * NBUC, np.int64)
        st[1:] = np.cumsum(cnts)[:-1]
        rank = np.arange(len(dslot)) - st[flat]
        spos = off_jb[blk, buc] + rank  # stream row of each edge
        idxs = np.zeros(NIDX, np.int16)
        idxs[spos] = (sg - buc * BUC).astype(np.int16)
        # one-hot column: edge's (chunk, block) pair
        dstloc = np.full((P, NCHD), 255.0, ml_dtypes.bfloat16)
        ech = spos // P  # stream chunk of edge
        # dc = dstcol_jb[j,b] + (ech - first chunk of (j,b) range)
        first_ch = off_jb[blk, buc] // P
        dc = dstcol_jb[blk, buc] + (ech - first_ch)
        dstloc[spos % P, dc] = (dslot % P).astype(np.float32)

        idx16 = idxs.reshape(NIDX // 16, 16).T.copy()  # [16, NIDX/16]
        idx_full = np.tile(idx16, (8, 1))  # [128, NIDX/16]

        lo = c * NPC
        invd = 1.0 / np.maximum(deg, 1.0)
        nodes = np.arange(lo, min((c + 1) * NPC, N))
        node_of_slot = np.full(SLOTS, -1, np.int64)
        node_of_slot[perm[nodes]] = nodes
        invd_slot = np.ones(SLOTS, np.float32)
        real = node_of_slot >= 0
        invd_slot[real] = invd[node_of_slot[real]].astype(np.float32)
        invd_sb = invd_slot.reshape(NBLK, P).T.copy()  # [P, NBLK]

        x_slot = np.zeros(SLOTS, np.int64)
        x_slot[perm[nodes]] = x[nodes]
        xg = x_slot.astype(np.int16)  # emb gather stream (slot order)
        xg16 = np.tile(xg.reshape(SLOTS // 16, 16).T.copy(), (8, 1))

        per_core.append(
            {
                "gidx": idx_full,
                "dstloc": dstloc,
                "invd": invd_sb,
                "invd0": invd_sb / EMB_SCALE,
                "xgidx": xg16,
                "c8": c8,
            }
        )

    meta = {
        "NPC": NPC,
        "SLOTS": SLOTS,
        "NBLK": NBLK,
        "HBLK": HBLK,
        "HS": HS,
        "TBL": TBL,
        "BUC": BUC,
        "groups": groups,
        "nh0": nh0,
        "calls": calls,
        "blockmm": blockmm,
        "dcol0": dcol0,
        "NCH": NCH,
        "NCHD": NCHD,
        "NIDX": NIDX,
        "gslot": gslot,
    }
    return per_core, meta


def _build_program(meta, V, L, single_core=False):
    SLOTS, NBLK, TBL, BUC = meta["SLOTS"], meta["NBLK"], meta["TBL"], meta["BUC"]
    HBLK, HS = meta["HBLK"], meta["HS"]
    groups, nh0, calls, blockmm = (
        meta["groups"],
        meta["nh0"],
        meta["calls"],
        meta["blockmm"],
    )
    dcol0 = meta["dcol0"]
    NCH, NCHD, NIDX = meta["NCH"], meta["NCHD"], meta["NIDX"]
    CBMAX = max(nch for gc in calls for (_, _, nch, _) in gc)

    f32, f32r, bf16 = mybir.dt.float32, mybir.dt.float32r, mybir.dt.bfloat16
    i16, f8 = mybir.dt.int16, mybir.dt.float8e4

    nc = bacc.Bacc(
        "TRN2",
        target_bir_lowering=False,
        debug=False,
        enable_asserts=False,
        num_devices=1 if single_core else M,
        num_swdge_queues=NQ,
    )

    gidx_d = nc.dram_tensor("gidx", [P, NIDX // 16], i16, kind="ExternalInput")
    dstloc_d = nc.dram_tensor("dstloc", [P, NCHD], bf16, kind="ExternalInput")
    invd_d = nc.dram_tensor("invd", [P, NBLK], f32, kind="ExternalInput")
    invd0_d = nc.dram_tensor("invd0", [P, NBLK], f32, kind="ExternalInput")
    emb8_d = nc.dram_tensor("emb8", [P, VT * D], f8, kind="ExternalInput")
    c8_d = nc.dram_tensor("c8", [NBLK, P, VT * P], f8, kind="ExternalInput")
    xgidx_d = nc.dram_tensor("xgidx", [P, SLOTS // 16], i16, kind="ExternalInput")
    emb16_d = nc.dram_tensor("emb16", [V, D], bf16, kind="ExternalInput")
    ws_d = nc.dram_tensor("ws", [L, D, D], f32, kind="ExternalInput")
    wn_d = nc.dram_tensor("wn", [L, D, D], f32, kind="ExternalInput")
    bias_d = nc.dram_tensor("bias", [L, D], f32, kind="ExternalInput")
    hout_d = nc.dram_tensor("hout", [SLOTS, D], f32, kind="ExternalOutput")

    h_shard = nc.dram_tensor("h_shard", [SLOTS, D], bf16, kind="Internal")
    # double-buffered global table: layer l gathers from tab[l%2] while
    # its output AllGathers into tab[(l+1)%2] (no WAR on the live table)
    h_full_t = [
        nc.dram_tensor(
            f"h_full{t}", [TBL, D], bf16, kind="Internal", addr_space="Shared"
        )
        for t in range(2)
    ]

    rg = [list(range(M))]
    qrr = [0]  # gather queue round-robin counter

    with tile.TileContext(nc) as tc:
        with (
            tc.tile_pool(name="const", bufs=1) as cpool,
            tc.tile_pool(name="state", bufs=1) as spool,
            tc.tile_pool(name="gath", bufs=8) as gpool,
            tc.tile_pool(name="oh", bufs=7) as ohpool,
            tc.tile_pool(name="c8", bufs=2) as c8pool,
            tc.tile_pool(name="fm", bufs=2) as fmpool,
            tc.tile_pool(name="small", bufs=2) as smpool,
            tc.tile_pool(name="ps_blk", bufs=3, space="PSUM") as ps_blk,
            tc.tile_pool(name="ps_nm", bufs=2, space="PSUM") as ps_nm,
            tc.tile_pool(name="ps_d", bufs=1, space="PSUM") as ps_d,
        ):
            # ---- constants ----
            ident_f = cpool.tile([P, P], f32, tag="ident_f")
            make_identity(nc, ident_f[:])

            # iota repeated 8x along free dim: iota8[p, r*128 + c] = c
            iota8 = cpool.tile([P, 8 * P], bf16, tag="iota8")
            nc.gpsimd.iota(
                iota8[:].rearrange("p (r c) -> p r c", c=P),
                pattern=[[0, 8], [1, P]],
                base=0,
                channel_multiplier=0,
                allow_small_or_imprecise_dtypes=True,
            )

            gidx_sb = cpool.tile([P, NIDX // 16], i16, tag="gidx")
            nc.sync.dma_start(gidx_sb[:], gidx_d[:, :])
            dstloc_sb = cpool.tile([P, NCHD], bf16, tag="dstloc")
            nc.sync.dma_start(dstloc_sb[:], dstloc_d[:, :])
            invd_sb = cpool.tile([P, NBLK], f32, tag="invd")
            nc.sync.dma_start(invd_sb[:], invd_d[:, :])
            invd0_sb = cpool.tile([P, NBLK], f32, tag="invd0")
            nc.sync.dma_start(invd0_sb[:], invd0_d[:, :])
            emb8_sb = cpool.tile([P, VT * D], f8, tag="emb8")
            nc.sync.dma_start(emb8_sb[:], emb8_d[:, :])
            xg_sb = cpool.tile([P, SLOTS // 16], i16, tag="xgidx")
            nc.sync.dma_start(xg_sb[:], xgidx_d[:, :])

            w_sb = []
            for l in range(L):
                wsf = cpool.tile([P, D], f32, tag=f"wsf{l}")
                wnf = cpool.tile([P, D], f32, tag=f"wnf{l}")
                nc.sync.dma_start(wsf[:], ws_d[l, :, :])
                nc.sync.dma_start(wnf[:], wn_d[l, :, :])
                ws = cpool.tile([P, D], f32r, tag=f"ws{l}")
                wn = cpool.tile([P, D], f32r, tag=f"wn{l}")
                nc.scalar.copy(ws[:], wsf[:])
                nc.scalar.copy(wn[:], wnf[:])
                w_sb.append((ws, wn))
            b_sb = cpool.tile([P, L], f32, tag="bias")
            for l in range(L):
                nc.sync.dma_start(b_sb[:, l : l + 1], bias_d[l, :, None])

            # ---- embedding lookup: 4 dma_gather calls, one per queue ----
            e_sb = spool.tile([P, NBLK * D], bf16, tag="e")
            ev = e_sb[:].rearrange("p (j f) -> p j f", f=D)
            jsplit = [0, 25, 50, 75, NBLK]
            for qi in range(4):
                j0, j1 = jsplit[qi], jsplit[qi + 1]
                ni = (j1 - j0) * P
                nc.gpsimd.dma_gather(
                    ev[:, j0:j1, :],
                    emb16_d[:, :],
                    xg_sb[:, j0 * 8 : j1 * 8],
                    ni,
                    ni,
                    D,
                    single_packet=False,
                    queue_num=qi % NQ,
                )

            h_sb = spool.tile([P, NBLK * D], f32, tag="h")

            # zero-init gather ring buffers: ungathered tail rows of a
            # segment's last chunk must not be NaN (NaN * 0 = NaN)
            for _ in range(8):
                gz = gpool.tile([P, CBMAX, D], bf16, tag="gath")
                nc.vector.memset(gz[:, :, :], 0.0)

            shard_v = h_shard.ap().rearrange("(j p) f -> p j f", p=P)

            def store_half(src_tile, h, tab):
                j0, j1 = h * HBLK, (h + 1) * HBLK
                sv = src_tile[:, j0 * D : j1 * D].rearrange(
                    "p (j f) -> p j f", f=D
                )
                nc.gpsimd.dma_start(out=shard_v[:, j0:j1, :], in_=sv)  # cast
                if single_core:
                    return
                nc.gpsimd.collective_compute(
                    "AllGather",
                    mybir.AluOpType.bypass,
                    replica_groups=rg,
                    ins=[h_shard[h * HS : (h + 1) * HS, :]],
                    outs=[tab[h * M * HS : (h + 1) * M * HS, :]],
                )

            # no initial e-table AllGather: layer 0 aggregates from the
            # vocab count-matrices, so nothing reads a table until layer 1

            # ---- layers ----
            for l in range(L):
                cur = e_sb if l == 0 else h_sb
                h_full = h_full_t[l % 2]
                ws, wn = w_sb[l]
                gtiles = {gi: {} for gi in range(len(groups))}

                def issue(gi, sel01):
                    # issue this group's bucket-0/1 (sel01) or 2/3 calls
                    for (b, ch0, nch, ni) in calls[gi]:
                        if (b < 2) != sel01:
                            continue
                        gt = gpool.tile([P, CBMAX, D], bf16, tag="gath")
                        nc.gpsimd.dma_gather(
                            gt[:, 0:nch, :],
                            h_full[b * BUC : (b + 1) * BUC, :],
                            gidx_sb[:, ch0 * 8 : (ch0 + nch) * 8],
                            ni,
                            ni,
                            D,
                            single_packet=False,
                            queue_num=qrr[0] % NQ,
                        )
                        qrr[0] += 1
                        gtiles[gi][b] = (gt, ch0)

                if l > 0:
                    # prefetch bucket-0/1 of the first two groups so the
                    # Pool engine has work while collective-1 is in flight
                    issue(0, True)
                    issue(1, True)
                for gi, grp in enumerate(groups):
                    if l > 0:
                        if gi + 2 < len(groups):
                            issue(gi + 2, True)
                        issue(gi, False)
                    nfm = fmpool.tile([P, GRP * D], f32r, tag="nfm")
                    hfm = fmpool.tile([P, GRP * D], f32r, tag="hfm")
                    for bi, j in enumerate(grp):
                        blk_ps = ps_blk.tile([P, 2 * P], f32, tag="blk")
                        pa, pt = blk_ps[:, 0:P], blk_ps[:, P : 2 * P]
                        if l == 0:
                            # layer 0: neigh-sum = emb8^T @ count-matrix
                            # (no gathers: sources live in the 5k vocab)
                            cb = c8pool.tile([P, VT * P], f8, tag="c8")
                            nc.sync.dma_start(cb[:], c8_d[j, :, :])
                            for t in range(VT):
                                nc.tensor.matmul(
                                    pa,
                                    emb8_sb[:, t * D : (t + 1) * D],
                                    cb[:, t * P : (t + 1) * P],
                                    start=(t == 0),
                                    stop=(t == VT - 1),
                                )
                        else:
                            mms = blockmm[j]
                            nmm = len(mms)
                            # one-hot selectors, 8 columns per DVE op
                            # (1-port mode: never locks GpSimd out of
                            # the SBUF ports)
                            noct = (nmm + 7) // 8
                            d0 = int(dcol0[j])
                            ohqs = []
                            for t in range(noct):
                                ohq = ohpool.tile([P, 8 * P], bf16, tag="oh")
                                s = d0 + 8 * t
                                nc.vector.tensor_tensor(
                                    out=ohq[:].rearrange(
                                        "p (r c) -> p r c", c=P
                                    ),
                                    in0=dstloc_sb[:, s : s + 8].to_broadcast(
                                        [P, 8, P]
                                    ),
                                    in1=iota8[:].rearrange(
                                        "p (r c) -> p r c", c=P
                                    ),
                                    op=mybir.AluOpType.is_equal,
                                )
                                ohqs.append(ohq)
                            for ci, (b, sp, dc) in enumerate(mms):
                                gt, ch0 = gtiles[gi][b]
                                q, r = (dc - d0) // 8, (dc - d0) % 8
                                nc.tensor.matmul(
                                    pa,
                                    gt[:, sp - ch0, :],
                                    ohqs[q][:, r * P : (r + 1) * P],
                                    start=(ci == 0),
                                    stop=(ci == nmm - 1),
                                )
                        # pa is feature-major sum-aggregated neigh
                        nc.scalar.copy(nfm[:, bi * D : (bi + 1) * D], pa)
                        if l == 0:
                            cur_f = smpool.tile([P, D], f32, tag="curf")
                            nc.scalar.copy(cur_f[:], cur[:, j * D : (j + 1) * D])
                            nc.tensor.transpose(pt, cur_f[:], ident_f[:])
                        else:
                            nc.tensor.transpose(
                                pt, cur[:, j * D : (j + 1) * D], ident_f[:]
                            )
                        nc.scalar.copy(hfm[:, bi * D : (bi + 1) * D], pt)
                    gw = len(grp) * D
                    d_ps = ps_d.tile([P, 2 * GRP * D], f32, tag="d")
                    pdS = d_ps[:, 0 : GRP * D]
                    pdN = d_ps[:, GRP * D : 2 * GRP * D]
                    nc.tensor.matmul(
                        pdS[:, 0:gw], ws[:], hfm[:, 0:gw], start=True, stop=True
                    )
                    nc.tensor.matmul(
                        pdN[:, 0:gw], wn[:], nfm[:, 0:gw], start=True, stop=True
                    )
                    # self path + bias (feature-major, per-partition bias)
                    hbias = fmpool.tile([P, GRP * D], f32, tag="hbias")
                    nc.scalar.activation(
                        hbias[:, 0:gw],
                        pdS[:, 0:gw],
                        mybir.ActivationFunctionType.Identity,
                        bias=b_sb[:, l : l + 1],
                    )
                    nden = fmpool.tile([P, GRP * D], f32, tag="nden")
                    nc.scalar.copy(nden[:, 0:gw], pdN[:, 0:gw])
                    for bi, j in enumerate(grp):
                        nm_ps = ps_nm.tile([P, 2 * P], f32, tag="nm")
                        pnS, pnN = nm_ps[:, 0:P], nm_ps[:, P : 2 * P]
                        nc.tensor.transpose(
                            pnS, hbias[:, bi * D : (bi + 1) * D], ident_f[:]
                        )
                        nc.tensor.transpose(
                            pnN, nden[:, bi * D : (bi + 1) * D], ident_f[:]
                        )
                        # node-major: neigh * (1/deg), + self, ReLU
                        inv_l = invd0_sb if l == 0 else invd_sb
                        tn = smpool.tile([P, D], f32, tag="tn")
                        nc.scalar.activation(
                            tn[:],
                            pnN,
                            mybir.ActivationFunctionType.Identity,
                            scale=inv_l[:, j : j + 1],
                        )
                        hp = smpool.tile([P, D], f32, tag="hp")
                        nc.vector.tensor_tensor(
                            out=hp[:], in0=pnS, in1=tn[:],
                            op=mybir.AluOpType.add,
                        )
                        hr = smpool.tile([P, D], f32, tag="hr")
                        sq_ss = smpool.tile([P, 1], f32, tag="ss")
                        nc.scalar.activation(
                            hr[:], hp[:], mybir.ActivationFunctionType.Relu
                        )
                        # squares land in hp (dead after the ReLU) — only
                        # the accumulated sum-of-squares is consumed
                        nc.scalar.activation(
                            hp[:],
                            hr[:],
                            mybir.ActivationFunctionType.Square,
                            accum_out=sq_ss[:],
                        )
                        nrm = smpool.tile([P, 1], f32, tag="nrm")
                        nc.scalar.sqrt(nrm[:], sq_ss[:])
                        nc.vector.tensor_scalar_max(nrm[:], nrm[:], 1e-12)
                        inv = smpool.tile([P, 1], f32, tag="inv")
                        nc.vector.reciprocal(inv[:], nrm[:])
                        htmp = smpool.tile([P, D], f32, tag="htmp")
                        nc.vector.tensor_tensor(
                            out=htmp[:],
                            in0=hr[:],
                            in1=inv[:, 0:1].to_broadcast([P, D]),
                            op=mybir.AluOpType.mult,
                        )
                        nc.vector.tensor_tensor(
                            out=h_sb[:, j * D : (j + 1) * D],
                            in0=htmp[:],
                            in1=e_sb[:, j * D : (j + 1) * D],
                            op=mybir.AluOpType.add,
                        )
                    if l < L - 1 and gi == nh0 - 1:
                        store_half(h_sb, 0, h_full_t[(l + 1) % 2])
                if l < L - 1:
                    store_half(h_sb, 1, h_full_t[(l + 1) % 2])

            hout_v = hout_d.ap().rearrange("(j p) f -> p j f", p=P)
            h_v = h_sb[:].rearrange("p (j f) -> p j f", f=D)
            nc.sync.dma_start(hout_v, h_v)

    nc.compile()
    return nc


def kernel(x, src, dst, emb, Ws, Wn, b, _trace=False):
    x = np.asarray(x)
    src = np.asarray(src)
    dst = np.asarray(dst)
    emb = np.ascontiguousarray(np.asarray(emb, dtype=np.float32))
    Ws = np.ascontiguousarray(np.asarray(Ws, dtype=np.float32))
    Wn = np.ascontiguousarray(np.asarray(Wn, dtype=np.float32))
    b = np.ascontiguousarray(np.asarray(b, dtype=np.float32))
    N = x.shape[0]
    V, _ = emb.shape
    L = Ws.shape[0]

    per_core, meta = _host_prep(x, src, dst, N)
    nc = _build_program(meta, V, L)

    # emb, upscaled for fp8 and pre-swizzled to the SBUF tile layout:
    # emb8s[p, t*D + f] = (emb * EMB_SCALE)[t*P + p, f]
    embp = np.zeros((VP, D), np.float32)
    embp[:V] = emb * EMB_SCALE
    emb8s = np.ascontiguousarray(
        embp.reshape(VT, P, D).transpose(1, 0, 2).reshape(P, VT * D)
    ).astype(ml_dtypes.float8_e4m3fn)
    emb16 = np.ascontiguousarray(emb.astype(ml_dtypes.bfloat16))

    in_maps = []
    for c in range(M):
        pc = per_core[c]
        in_maps.append(
            {
                "gidx": np.ascontiguousarray(pc["gidx"]),
                "dstloc": np.ascontiguousarray(pc["dstloc"]),
                "invd": np.ascontiguousarray(pc["invd"]),
                "invd0": np.ascontiguousarray(pc["invd0"]),
                "xgidx": np.ascontiguousarray(pc["xgidx"]),
                "c8": np.ascontiguousarray(pc["c8"]),
                "emb8": emb8s,
                "emb16": emb16,
                "ws": Ws,
                "wn": Wn,
                "bias": b,
            }
        )

    res = bass_utils.run_bass_kernel_spmd(
        nc, in_maps, core_ids=list(range(M)), trace=_trace
    )
    global LAST_EXEC_NS
    LAST_EXEC_NS = res.exec_time_ns
    outs = [np.asarray(r["hout"], dtype=np.float32) for r in res.results]
    big = np.concatenate(outs, axis=0)
    return big[meta["gslot"]]
